# revision 31
# baseline (speedup 1.0000x reference)
"""GAT layer (single head) on 8 Trainium2 NeuronCores.

Strategy: destination-sharded edge parallelism, phase-pipelined.
  - Nodes padded to NPAD = 8*NB*128; core k owns NB blocks of 128 nodes.
  - Host sorts edges by (dst-core, src-chunk, dst-block, src-row) and pads
    each (block, chunk) run to whole tiles of 128 edges (capacity C tiles,
    the max over all runs). src-chunk = src // (NPAD/4) (column range), so
    the node table for chunk q is complete after the q-th quarter of
    phase 1; sorting by src-row within a run improves gather locality.
  - Device, per core:
      prepass: WAUG = [W.T | wl | wr]; er for the core's OWN nodes only
               (h slice @ wr), then R = exp(-0.8*er) broadcast to all
               partitions as a replicated block-major table R_rep.
      phase 1 (chunk q): zaug[n] = [z | el | er | 1 | A | r] (bf16,
               A = exp(el), r = exp(-0.8*el)) via one fused matmul with
               WAUG; 256B rows to the chunk-q DRAM table.
      phase 2 (chunk q): per edge tile of 128: dma_gather zaug[src]
               (4 SWDGE queues, 2048-descriptor ring). The per-dst-node
               factor exp(er) cancels in the softmax ratio, so
                   ex_eff[e, n] = A_e * max(1, r_e * R_n)
               equals exp(leaky_relu(el+er, 0.2)) / exp(er) exactly.
               Tile work (bf16): M = max(r_e*R_rep[b], 1);
               oh = (iota==dl)*A_e*M; Y[b] += [z|..|1].T @ oh in PSUM per
               (block, chunk); acc[b] += Y (f32, numerator rows 0:32,
               denominator row 34 via the ones column).
      Phase 1 chunk q+1 is emitted interleaved with phase 2 chunk q, so
      gathers and edge compute overlap table construction. Finally
      out = num / max(denom, eps) in f32. Softmax max-subtraction is
      dropped: |e| stays small for this model, so exp() is
      well-conditioned and the softmax ratio is unchanged.

  DRAM chunk tables use a tiled layout: node n (chunk-local l) lives at
  row (l % 128) * (NPAD/512) + l // 128 of table (n // (NPAD/4)), so
  phase 1 writes them with contiguous per-partition DMA runs; the host
  bakes this mapping into the gather indices.
"""

import sys

sys.path.insert(0, "/opt/trn_rl_repo")

import numpy as np
import ml_dtypes

import concourse.bacc as bacc
import concourse.bass as bass
import concourse.tile as tile
from concourse import mybir
from concourse.bass_utils import run_bass_kernel_spmd
from concourse.masks import make_identity

F32 = mybir.dt.float32
BF16 = mybir.dt.bfloat16
I16 = mybir.dt.int16

N_NODES = 100000
IN_FEATS = 128
OUT_FEATS = 32
NEG_SLOPE = 0.2
N_CORES = 8
BLK = 128
NB = 98  # blocks per core (full problem)
EL = 128  # table row: 128 bf16 = 256B (dma_gather granularity)
NQ = 4  # chunks of the z table (by node column range)
ZG = 512  # nodes per z-phase group
BGS = 7  # blocks per phase-2 gather group

C_EL = 32  # el column in zaug row
C_ER = 33  # er column
C_ONE = 34  # constant-one column
C_A = 35  # A = exp(el)
C_R = 36  # r = exp(-0.8*el)

_cache = {}
SIM_INIT = False  # set True when running under CoreSim (full-init for checker)
N_QUEUES = 4  # SWDGE queues to round-robin gathers over (1..4)
SCRATCH = 32768  # dynamic_dma_scratch_size (ring bytes; 16B/descriptor)
SKIP_GATHER = False  # timing probe: skip dma_gather (breaks correctness)
SKIP_P2C = False  # timing probe: skip phase-2 DVE/PE compute

_qctr = [0]


def _qrr():
    """Round-robin SWDGE queue assignment."""
    q = _qctr[0] % N_QUEUES
    _qctr[0] += 1
    return q


def _build(C, nb=NB, bgs=BGS):
    """C = tiles of 128 edges per (block, chunk) run."""
    assert nb % bgs == 0, (nb, bgs)
    core_nodes = nb * BLK
    npad = N_CORES * core_nodes
    ncols = npad // BLK
    chunk_nodes = npad // NQ
    chunk_cols = ncols // NQ  # 196
    chunk_rows = chunk_nodes  # rows per chunk table
    gpc = chunk_nodes // ZG  # phase-1 groups per chunk (49)
    sub = ZG // BLK
    assert chunk_rows < 32768 and core_nodes < 32768
    T = NQ * nb * C  # tile columns per core
    NW = T * BLK // 16  # wrapped-index columns
    NGB = nb // bgs  # gather groups per chunk (14)
    NCOL = bgs * C  # tile columns per gather group

    _qctr[0] = 0
    nc = bacc.Bacc("TRN2", target_bir_lowering=False, debug=False,
                   num_devices=N_CORES, num_swdge_queues=N_QUEUES,
                   dynamic_dma_scratch_size=SCRATCH)

    hT = nc.dram_tensor("hT", [IN_FEATS, npad], BF16, kind="ExternalInput")
    Wt = nc.dram_tensor("Wt", [OUT_FEATS, IN_FEATS], F32, kind="ExternalInput")
    av = nc.dram_tensor("av", [2 * OUT_FEATS, 1], F32, kind="ExternalInput")
    srcw = nc.dram_tensor("srcw", [BLK, NW], I16, kind="ExternalInput")
    dstloc = nc.dram_tensor("dstloc", [BLK, T], F32, kind="ExternalInput")
    out = nc.dram_tensor("out", [core_nodes, OUT_FEATS], F32,
                         kind="ExternalOutput")

    zaug = [nc.dram_tensor(f"zaug{q}", [chunk_rows, EL], BF16)
            for q in range(NQ)]
    rrow = nc.dram_tensor("rrow", [1, core_nodes], BF16)  # R flatten bounce

    hTv = hT.ap().rearrange("f (k n) -> f k n", k=N_CORES)

    with tile.TileContext(nc) as tc:
        with tc.tile_pool(name="const", bufs=1) as cpool:
            ident = cpool.tile([128, 128], F32)
            make_identity(nc, ident[:])
            identb = cpool.tile([128, 128], BF16)
            nc.vector.tensor_copy(out=identb[:], in_=ident[:])
            iota = cpool.tile([128, BLK], BF16)
            nc.gpsimd.iota(iota[:], pattern=[[1, BLK]], base=0,
                           channel_multiplier=0,
                           allow_small_or_imprecise_dtypes=True)
            ones1 = cpool.tile([1, BLK], BF16)
            nc.vector.memset(ones1[:], 1.0)

            # WAUG = [W.T | wl | wr]  (wl = W.T a_l, wr = W.T a_r)
            waug = cpool.tile([IN_FEATS, C_ONE], BF16)
            nc.vector.memset(waug[:], 0.0)
            with tc.tile_pool(name="wprep", bufs=1) as wpool, \
                 tc.tile_pool(name="wpsum", bufs=2, space="PSUM") as wps:
                w_sb = wpool.tile([OUT_FEATS, IN_FEATS], F32)
                nc.sync.dma_start(out=w_sb[:], in_=Wt[:])
                al_sb = wpool.tile([OUT_FEATS, 1], F32)
                nc.sync.dma_start(out=al_sb[:], in_=av[0:OUT_FEATS, :])
                ar_sb = wpool.tile([OUT_FEATS, 1], F32)
                nc.sync.dma_start(out=ar_sb[:],
                                  in_=av[OUT_FEATS:2 * OUT_FEATS, :])
                wt_ps = wps.tile([IN_FEATS, OUT_FEATS], F32)
                nc.tensor.transpose(out=wt_ps[:], in_=w_sb[:],
                                    identity=ident[0:OUT_FEATS, 0:OUT_FEATS])
                nc.vector.tensor_copy(out=waug[:, 0:OUT_FEATS], in_=wt_ps[:])
                wl_ps = wps.tile([IN_FEATS, 1], F32)
                nc.tensor.matmul(out=wl_ps[:], lhsT=w_sb[:],
                                 rhs=al_sb[:], start=True, stop=True)
                nc.vector.tensor_copy(out=waug[:, C_EL:C_EL + 1],
                                      in_=wl_ps[:])
                wr_ps = wps.tile([IN_FEATS, 1], F32)
                nc.tensor.matmul(out=wr_ps[:], lhsT=w_sb[:],
                                 rhs=ar_sb[:], start=True, stop=True)
                nc.vector.tensor_copy(out=waug[:, C_ER:C_ER + 1],
                                      in_=wr_ps[:])

            # ---- prepass: er for OWN nodes only -> R_rep (replicated) ----
            pid = nc.gpsimd.partition_id()
            r_rep = cpool.tile([128, nb * BLK], BF16)
            with tc.tile_pool(name="rprep", bufs=1) as rpool2, \
                 tc.tile_pool(name="rh", bufs=3) as rhpool, \
                 tc.tile_pool(name="rps", bufs=2, space="PSUM") as rps:
                er_loc = rpool2.tile([BLK, nb], F32)
                RZG = 7 * BLK  # 896 columns per own-h group (nb % 7 == 0)
                for j0 in range(0, core_nodes, RZG):
                    h2 = rhpool.tile([IN_FEATS, 1, RZG], BF16, tag="h2",
                                     name="h2")
                    nc.gpsimd.dma_start(
                        out=h2[:],
                        in_=hTv[:, bass.ts(pid, 1), j0:j0 + RZG])
                    for s in range(RZG // BLK):
                        blk = j0 // BLK + s
                        e_ps = rps.tile([BLK, 1], F32, tag="ep", name="e_ps")
                        nc.tensor.matmul(
                            out=e_ps[:],
                            lhsT=h2[:, 0, s * BLK:(s + 1) * BLK],
                            rhs=waug[:, C_ER:C_ER + 1],
                            start=True, stop=True)
                        nc.scalar.copy(out=er_loc[:, blk:blk + 1],
                                       in_=e_ps[:])
                r_loc = rpool2.tile([BLK, nb], BF16)
                nc.scalar.activation(out=r_loc[:], in_=er_loc[:],
                                     func=mybir.ActivationFunctionType.Exp,
                                     scale=-(1.0 - NEG_SLOPE))
                rt_ps = rps.tile([nb, BLK], BF16)
                nc.tensor.transpose(out=rt_ps[:], in_=r_loc[:],
                                    identity=identb[:])
                rt_sb = rpool2.tile([nb, BLK], BF16)
                nc.vector.tensor_copy(out=rt_sb[:], in_=rt_ps[:])
                nc.sync.dma_start(
                    out=rrow.ap().rearrange("o (b f) -> (o b) f", b=nb),
                    in_=rt_sb[:])
                r_flat = rpool2.tile([1, core_nodes], BF16)
                nc.sync.dma_start(out=r_flat[:], in_=rrow[:])
                for j0 in range(0, core_nodes, 512):
                    j1 = min(j0 + 512, core_nodes)
                    rb_ps = rps.tile([128, j1 - j0], F32, tag="rb",
                                     name="rb_ps")
                    nc.tensor.matmul(out=rb_ps[:], lhsT=ones1[:],
                                     rhs=r_flat[:, j0:j1],
                                     start=True, stop=True)
                    nc.vector.tensor_copy(out=r_rep[:, j0:j1], in_=rb_ps[:])

            # ---- interleaved phase 1 (table build) + phase 2 (edges) ----
            with tc.tile_pool(name="ix", bufs=1) as ixpool:
                srcw_sb = ixpool.tile([BLK, NW], I16)
                nc.sync.dma_start(out=srcw_sb[:], in_=srcw[:])
                dl_sb = ixpool.tile([BLK, T], F32)
                nc.sync.dma_start(out=dl_sb[:], in_=dstloc[:])

                with tc.tile_pool(name="zh", bufs=3) as hpool, \
                     tc.tile_pool(name="zrow", bufs=1) as zrpool, \
                     tc.tile_pool(name="zps", bufs=2, space="PSUM") as zps, \
                     tc.tile_pool(name="zg", bufs=4) as zgpool, \
                     tc.tile_pool(name="ar", bufs=4) as arpool, \
                     tc.tile_pool(name="m", bufs=4) as mpool, \
                     tc.tile_pool(name="oh", bufs=4) as ohpool, \
                     tc.tile_pool(name="acc", bufs=1) as apool, \
                     tc.tile_pool(name="yps", bufs=4, space="PSUM") as ypool, \
                     tc.tile_pool(name="ytp", bufs=2, space="PSUM") as ytpool, \
                     tc.tile_pool(name="fin", bufs=6) as fpool, \
                     tc.tile_pool(name="ost", bufs=2) as opool:
                    NY = C_ONE + 1
                    # pinned zrows buffers: ones column set once
                    ZRB = 3
                    zrows_bufs = []
                    for i in range(ZRB):
                        zr = zrpool.tile([128, sub, EL], BF16, tag=f"zr{i}",
                                         name=f"zrows{i}")
                        if SIM_INIT:
                            nc.vector.memset(zr[:, :, C_R + 1:], 0.0)
                        nc.vector.memset(zr[:, :, C_ONE:C_ONE + 1], 1.0)
                        zrows_bufs.append(zr)
                    acc = apool.tile([NY, nb, BLK], F32)
                    nc.vector.memset(acc[:], 0.0)

                    def phase1_chunk(q):
                        zaug_t = zaug[q].ap().rearrange(
                            "(p c) z -> p c z", p=BLK)
                        for mm in range(gpc):
                            g = q * gpc + mm
                            n0 = g * ZG
                            htile = hpool.tile([IN_FEATS, ZG], BF16,
                                               tag="ht", name="htile")
                            nc.sync.dma_start(out=htile[:],
                                              in_=hT[:, n0:n0 + ZG])
                            zrows = zrows_bufs[g % ZRB]
                            for s in range(sub):
                                z_ps = zps.tile([128, C_ONE], F32, tag="zp",
                                                name="z_ps")
                                nc.tensor.matmul(
                                    out=z_ps[:],
                                    lhsT=htile[:, s * BLK:(s + 1) * BLK],
                                    rhs=waug[:], start=True, stop=True)
                                nc.scalar.copy(out=zrows[:, s, 0:C_ONE],
                                               in_=z_ps[:])
                            nc.scalar.activation(
                                out=zrows[:, :, C_A],
                                in_=zrows[:, :, C_EL],
                                func=mybir.ActivationFunctionType.Exp)
                            nc.scalar.activation(
                                out=zrows[:, :, C_R],
                                in_=zrows[:, :, C_EL],
                                func=mybir.ActivationFunctionType.Exp,
                                scale=-(1.0 - NEG_SLOPE))
                            if SIM_INIT:
                                nc.sync.dma_start(
                                    out=zaug_t[:, sub * mm:sub * (mm + 1), :],
                                    in_=zrows[:])
                            else:
                                # write only the 37 used columns of each
                                # 256B row (sliced rows cut HBM traffic 3.5x)
                                nc.sync.dma_start(
                                    out=zaug_t[:, sub * mm:sub * (mm + 1),
                                               0:C_R + 1],
                                    in_=zrows[:, :, 0:C_R + 1])

                    def phase2_chunk(q):
                        for bgi in range(NGB):
                            colbase = q * nb * C + bgi * NCOL
                            w0 = colbase * BLK // 16
                            zg = zgpool.tile([BLK, NCOL, EL], BF16,
                                             tag="zg", name="zg")
                            GCH = 8  # tile-columns per call (1024 idxs)
                            for j0 in range([] if SKIP_GATHER else range(0, NCOL, GCH)) if False else (range(0) if SKIP_GATHER else range(0, NCOL, GCH)):
                                j1 = min(j0 + GCH, NCOL)
                                ni = (j1 - j0) * BLK
                                wj = w0 + j0 * BLK // 16
                                nc.gpsimd.dma_gather(
                                    out_ap=zg[:, j0:j1, :],
                                    in_ap=zaug[q][:],
                                    idxs_ap=srcw_sb[:, wj:wj + ni // 16],
                                    num_idxs=ni, num_idxs_reg=ni,
                                    elem_size=EL,
                                    queue_num=_qrr())
                            if SKIP_GATHER:
                                nc.vector.memset(zg[:, 0:1, :], 0.0)
                            a_sb = arpool.tile([BLK, NCOL], F32, tag="a",
                                               name="a_sb")
                            nc.vector.tensor_copy(out=a_sb[:],
                                                  in_=zg[:, :, C_A])
                            rr_sb = arpool.tile([BLK, NCOL], F32, tag="rr",
                                                name="rr_sb")
                            nc.vector.tensor_copy(out=rr_sb[:],
                                                  in_=zg[:, :, C_R])
                            for p0 in range(0) if SKIP_P2C else range(0, bgs, 4):
                                pw = min(4, bgs - p0)
                                y_ps = ypool.tile([NY, 4, BLK], F32, tag="y",
                                                  name="y_ps")
                                for bi in range(pw):
                                    b = p0 + bi
                                    bb = bgi * bgs + b
                                    m_all = mpool.tile([BLK, C, BLK], BF16,
                                                       tag="m", name="m_t")
                                    oh_all = ohpool.tile([BLK, C, BLK], BF16,
                                                         tag="oh", name="oh")
                                    for t in range(C):
                                        lcol = b * C + t
                                        col = colbase + lcol
                                        nc.vector.tensor_scalar(
                                            oh_all[:, t, :], iota[:],
                                            dl_sb[:, col:col + 1],
                                            a_sb[:, lcol:lcol + 1],
                                            mybir.AluOpType.is_equal,
                                            mybir.AluOpType.mult)
                                        # m = oh * r_e * R_n (fused)
                                        nc.vector.scalar_tensor_tensor(
                                            out=m_all[:, t, :],
                                            in0=r_rep[:, bb * BLK:
                                                      (bb + 1) * BLK],
                                            scalar=rr_sb[:, lcol:lcol + 1],
                                            in1=oh_all[:, t, :],
                                            op0=mybir.AluOpType.mult,
                                            op1=mybir.AluOpType.mult)
                                    # oh = max(oh, oh*r*R) = oh*max(1, r*R)
                                    nc.vector.tensor_tensor(
                                        out=oh_all[:], in0=oh_all[:],
                                        in1=m_all[:],
                                        op=mybir.AluOpType.max)
                                    for t in range(C):
                                        lcol = b * C + t
                                        nc.tensor.matmul(
                                            out=y_ps[:, bi, :],
                                            lhsT=zg[:, lcol, 0:NY],
                                            rhs=oh_all[:, t, :],
                                            start=(t == 0),
                                            stop=(t == C - 1))
                                bb0 = bgi * bgs + p0
                                nc.vector.tensor_add(
                                    out=acc[:, bb0:bb0 + pw, :],
                                    in0=acc[:, bb0:bb0 + pw, :],
                                    in1=y_ps[:, 0:pw, :])

                    phase1_chunk(0)
                    for q in range(NQ):
                        if q + 1 < NQ:
                            phase1_chunk(q + 1)
                        phase2_chunk(q)

                    # ---- finalize: normalize and write out ----
                    OB = 14  # blocks per output group
                    for og in range(nb // OB):
                        ost = opool.tile([BLK, OB, OUT_FEATS], F32)
                        for b in range(OB):
                            bb = og * OB + b
                            yt = ytpool.tile([BLK, NY], F32)
                            nc.tensor.transpose(out=yt[:], in_=acc[:, bb, :],
                                                identity=ident[0:NY, 0:NY])
                            den = fpool.tile([BLK, 1], F32)
                            nc.vector.tensor_scalar(
                                den[:], yt[:, C_ONE:C_ONE + 1], 1e-16, None,
                                mybir.AluOpType.max)
                            rden = fpool.tile([BLK, 1], F32)
                            nc.vector.reciprocal(out=rden[:], in_=den[:])
                            nc.vector.tensor_scalar(
                                ost[:, b, :], yt[:, 0:OUT_FEATS], rden[:],
                                None, mybir.AluOpType.mult)
                        n0 = og * OB * BLK
                        nc.sync.dma_start(
                            out=out[n0:n0 + OB * BLK, :].rearrange(
                                "(s p) c -> p s c", p=BLK),
                            in_=ost[:])

    nc.compile()
    return nc


def _prep(h, W, a, src, dst, nb=NB, n_nodes=N_NODES):
    """Host-side sharding / index layout (integer index manipulation,
    zero-padding and dtype casts only - all floating-point math runs on
    device)."""
    core_nodes = nb * BLK
    npad = N_CORES * core_nodes
    chunk_nodes = npad // NQ
    chunk_cols = chunk_nodes // BLK

    h = np.asarray(h, dtype=np.float32)
    W = np.ascontiguousarray(np.asarray(W, dtype=np.float32))
    a = np.asarray(a, dtype=np.float32).reshape(-1)
    src = np.asarray(src, dtype=np.int64)
    dst = np.asarray(dst, dtype=np.int64)

    hT = np.zeros((IN_FEATS, npad), dtype=ml_dtypes.bfloat16)
    hT[:, :n_nodes] = h.T.astype(ml_dtypes.bfloat16)
    av = np.ascontiguousarray(a.reshape(-1, 1), dtype=np.float32)

    core = dst // core_nodes
    b_of = (dst % core_nodes) // BLK
    q_of = src // chunk_nodes
    grp = (core * NQ + q_of) * nb + b_of
    # chunk-local tiled table row of src
    loc = src - q_of * chunk_nodes
    src_t = (loc % BLK) * chunk_cols + loc // BLK
    # sort by (group, src-table-row): src-sorted runs improve gather locality
    order = np.argsort(grp * (1 << 24) + src_t, kind="stable")
    gs = grp[order]
    ds = dst[order]

    counts = np.bincount(gs, minlength=N_CORES * NQ * nb)
    C = int(max(1, -(-counts.max() // BLK)))
    T = NQ * nb * C
    NW = T * BLK // 16

    # global slot of each sorted edge
    starts = np.zeros(N_CORES * NQ * nb + 1, dtype=np.int64)
    np.cumsum(counts, out=starts[1:])
    rank = np.arange(len(gs)) - starts[gs]
    # within-core group index: (q * nb + b) for that core
    gloc = gs % (NQ * nb)
    slot = gloc * (C * BLK) + rank  # slot within the core's edge buffer

    src_i16 = src_t[order].astype(np.int16)
    dl_all = (ds % core_nodes - b_of[order] * BLK).astype(np.float32)

    srcw = np.zeros((N_CORES, BLK, NW), dtype=np.int16)
    dstloc = np.full((N_CORES, BLK, T), -1.0, dtype=np.float32)
    for k in range(N_CORES):
        m = core[order] == k
        sl = slot[m]
        sflat = np.zeros(T * BLK, dtype=np.int16)
        dflat = np.full(T * BLK, -1.0, dtype=np.float32)
        sflat[sl] = src_i16[m]
        dflat[sl] = dl_all[m]
        # wrapped-16, replicated over the 8 gpsimd groups
        srcw[k] = np.tile(sflat.reshape(-1, 16).T, (8, 1))
        dstloc[k] = dflat.reshape(T, BLK).T
    return hT, W, av, srcw, dstloc, C


def kernel(h, W, a, src, dst):
    hT, Wm, av, srcw, dstloc, C = _prep(h, W, a, src, dst)
    if C not in _cache:
        _cache[C] = _build(C)
    nc = _cache[C]
    in_maps = []
    for k in range(N_CORES):
        in_maps.append({
            "hT": hT,
            "Wt": Wm,
            "av": av,
            "srcw": srcw[k],
            "dstloc": dstloc[k],
        })
    global _last
    _last = (nc, in_maps)
    res = run_bass_kernel_spmd(nc, in_maps, list(range(N_CORES)))
    outs = [res.results[k]["out"] for k in range(N_CORES)]
    full = np.concatenate(outs, axis=0)[:N_NODES]
    return np.ascontiguousarray(full, dtype=np.float32)


_last = None


# revision 32
# speedup vs baseline: 1.0193x; 1.0193x over previous
"""GAT layer (single head) on 8 Trainium2 NeuronCores.

Strategy: destination-sharded edge parallelism, phase-pipelined.
  - Nodes padded to NPAD = 8*NB*128; core k owns NB blocks of 128 nodes.
  - Host sorts edges by (dst-core, src-chunk, dst-block, src-row) and pads
    each (block, chunk) run to whole tiles of 128 edges (capacity C tiles,
    the max over all runs). src-chunk = src // (NPAD/4) (column range), so
    the node table for chunk q is complete after the q-th quarter of
    phase 1; sorting by src-row within a run improves gather locality.
  - Device, per core:
      prepass: WAUG = [W.T | wl | wr]; er for the core's OWN nodes only
               (h slice @ wr), then R = exp(-0.8*er) broadcast to all
               partitions as a replicated block-major table R_rep.
      phase 1 (chunk q): zaug[n] = [z | el | er | 1 | A | r] (bf16,
               A = exp(el), r = exp(-0.8*el)) via one fused matmul with
               WAUG; 256B rows to the chunk-q DRAM table.
      phase 2 (chunk q): per edge tile of 128: dma_gather zaug[src]
               (4 SWDGE queues, 2048-descriptor ring). The per-dst-node
               factor exp(er) cancels in the softmax ratio, so
                   ex_eff[e, n] = A_e * max(1, r_e * R_n)
               equals exp(leaky_relu(el+er, 0.2)) / exp(er) exactly.
               Tile work (bf16): M = max(r_e*R_rep[b], 1);
               oh = (iota==dl)*A_e*M; Y[b] += [z|..|1].T @ oh in PSUM per
               (block, chunk); acc[b] += Y (f32, numerator rows 0:32,
               denominator row 34 via the ones column).
      Phase 1 chunk q+1 is emitted interleaved with phase 2 chunk q, so
      gathers and edge compute overlap table construction. Finally
      out = num / max(denom, eps) in f32. Softmax max-subtraction is
      dropped: |e| stays small for this model, so exp() is
      well-conditioned and the softmax ratio is unchanged.

  DRAM chunk tables use a tiled layout: node n (chunk-local l) lives at
  row (l % 128) * (NPAD/512) + l // 128 of table (n // (NPAD/4)), so
  phase 1 writes them with contiguous per-partition DMA runs; the host
  bakes this mapping into the gather indices.
"""

import sys

sys.path.insert(0, "/opt/trn_rl_repo")

import numpy as np
import ml_dtypes

import concourse.bacc as bacc
import concourse.bass as bass
import concourse.tile as tile
from concourse import mybir
from concourse.bass_utils import run_bass_kernel_spmd
from concourse.masks import make_identity

F32 = mybir.dt.float32
BF16 = mybir.dt.bfloat16
I16 = mybir.dt.int16

N_NODES = 100000
IN_FEATS = 128
OUT_FEATS = 32
NEG_SLOPE = 0.2
N_CORES = 8
BLK = 128
NB = 98  # blocks per core (full problem)
EL = 128  # table row: 128 bf16 = 256B (dma_gather granularity)
NQ = 4  # chunks of the z table (by node column range)
ZG = 512  # nodes per z-phase group
BGS = 7  # blocks per phase-2 gather group

C_EL = 32  # el column in zaug row
C_ER = 33  # er column
C_ONE = 34  # constant-one column
C_A = 35  # A = exp(el)
C_R = 36  # r = exp(-0.8*el)

_cache = {}
SIM_INIT = False  # set True when running under CoreSim (full-init for checker)
N_QUEUES = 4  # SWDGE queues to round-robin gathers over (1..4)
SCRATCH = 32768  # dynamic_dma_scratch_size (ring bytes; 16B/descriptor)
SKIP_GATHER = False  # timing probe: skip dma_gather (breaks correctness)
SKIP_P2C = False  # timing probe: skip phase-2 DVE/PE compute

_qctr = [0]


def _qrr():
    """Round-robin SWDGE queue assignment."""
    q = _qctr[0] % N_QUEUES
    _qctr[0] += 1
    return q


def _build(C, nb=NB, bgs=BGS):
    """C = tiles of 128 edges per (block, chunk) run."""
    assert nb % bgs == 0, (nb, bgs)
    core_nodes = nb * BLK
    npad = N_CORES * core_nodes
    ncols = npad // BLK
    chunk_nodes = npad // NQ
    chunk_cols = ncols // NQ  # 196
    chunk_rows = chunk_nodes  # rows per chunk table
    gpc = chunk_nodes // ZG  # phase-1 groups per chunk (49)
    sub = ZG // BLK
    assert chunk_rows < 32768 and core_nodes < 32768
    T = NQ * nb * C  # tile columns per core
    NW = T * BLK // 16  # wrapped-index columns
    NGB = nb // bgs  # gather groups per chunk (14)
    NCOL = bgs * C  # tile columns per gather group

    _qctr[0] = 0
    nc = bacc.Bacc("TRN2", target_bir_lowering=False, debug=False,
                   num_devices=N_CORES, num_swdge_queues=N_QUEUES,
                   dynamic_dma_scratch_size=SCRATCH)

    hT = nc.dram_tensor("hT", [IN_FEATS, npad], BF16, kind="ExternalInput")
    Wt = nc.dram_tensor("Wt", [OUT_FEATS, IN_FEATS], F32, kind="ExternalInput")
    av = nc.dram_tensor("av", [2 * OUT_FEATS, 1], F32, kind="ExternalInput")
    srcw = nc.dram_tensor("srcw", [BLK, NW], I16, kind="ExternalInput")
    dstloc = nc.dram_tensor("dstloc", [BLK, T], F32, kind="ExternalInput")
    out = nc.dram_tensor("out", [core_nodes, OUT_FEATS], F32,
                         kind="ExternalOutput")

    zaug = [nc.dram_tensor(f"zaug{q}", [chunk_rows, EL], BF16)
            for q in range(NQ)]
    rrow = nc.dram_tensor("rrow", [1, core_nodes], BF16)  # R flatten bounce

    hTv = hT.ap().rearrange("f (k n) -> f k n", k=N_CORES)

    with tile.TileContext(nc) as tc:
        with tc.tile_pool(name="const", bufs=1) as cpool:
            ident = cpool.tile([128, 128], F32)
            make_identity(nc, ident[:])
            identb = cpool.tile([128, 128], BF16)
            nc.vector.tensor_copy(out=identb[:], in_=ident[:])
            iota = cpool.tile([128, BLK], BF16)
            nc.gpsimd.iota(iota[:], pattern=[[1, BLK]], base=0,
                           channel_multiplier=0,
                           allow_small_or_imprecise_dtypes=True)
            ones1 = cpool.tile([1, BLK], BF16)
            nc.vector.memset(ones1[:], 1.0)

            # WAUG = [W.T | wl | wr]  (wl = W.T a_l, wr = W.T a_r)
            waug = cpool.tile([IN_FEATS, C_ONE], BF16)
            nc.vector.memset(waug[:], 0.0)
            with tc.tile_pool(name="wprep", bufs=1) as wpool, \
                 tc.tile_pool(name="wpsum", bufs=2, space="PSUM") as wps:
                w_sb = wpool.tile([OUT_FEATS, IN_FEATS], F32)
                nc.sync.dma_start(out=w_sb[:], in_=Wt[:])
                al_sb = wpool.tile([OUT_FEATS, 1], F32)
                nc.sync.dma_start(out=al_sb[:], in_=av[0:OUT_FEATS, :])
                ar_sb = wpool.tile([OUT_FEATS, 1], F32)
                nc.sync.dma_start(out=ar_sb[:],
                                  in_=av[OUT_FEATS:2 * OUT_FEATS, :])
                wt_ps = wps.tile([IN_FEATS, OUT_FEATS], F32)
                nc.tensor.transpose(out=wt_ps[:], in_=w_sb[:],
                                    identity=ident[0:OUT_FEATS, 0:OUT_FEATS])
                nc.vector.tensor_copy(out=waug[:, 0:OUT_FEATS], in_=wt_ps[:])
                wl_ps = wps.tile([IN_FEATS, 1], F32)
                nc.tensor.matmul(out=wl_ps[:], lhsT=w_sb[:],
                                 rhs=al_sb[:], start=True, stop=True)
                nc.vector.tensor_copy(out=waug[:, C_EL:C_EL + 1],
                                      in_=wl_ps[:])
                wr_ps = wps.tile([IN_FEATS, 1], F32)
                nc.tensor.matmul(out=wr_ps[:], lhsT=w_sb[:],
                                 rhs=ar_sb[:], start=True, stop=True)
                nc.vector.tensor_copy(out=waug[:, C_ER:C_ER + 1],
                                      in_=wr_ps[:])

            # ---- prepass: er for OWN nodes only -> R_rep (replicated) ----
            pid = nc.gpsimd.partition_id()
            r_rep = cpool.tile([128, nb * BLK], BF16)
            with tc.tile_pool(name="rprep", bufs=1) as rpool2, \
                 tc.tile_pool(name="rh", bufs=3) as rhpool, \
                 tc.tile_pool(name="rps", bufs=2, space="PSUM") as rps:
                er_loc = rpool2.tile([BLK, nb], F32)
                RZG = 7 * BLK  # 896 columns per own-h group (nb % 7 == 0)
                for j0 in range(0, core_nodes, RZG):
                    h2 = rhpool.tile([IN_FEATS, 1, RZG], BF16, tag="h2",
                                     name="h2")
                    nc.gpsimd.dma_start(
                        out=h2[:],
                        in_=hTv[:, bass.ts(pid, 1), j0:j0 + RZG])
                    for s in range(RZG // BLK):
                        blk = j0 // BLK + s
                        e_ps = rps.tile([BLK, 1], F32, tag="ep", name="e_ps")
                        nc.tensor.matmul(
                            out=e_ps[:],
                            lhsT=h2[:, 0, s * BLK:(s + 1) * BLK],
                            rhs=waug[:, C_ER:C_ER + 1],
                            start=True, stop=True)
                        nc.scalar.copy(out=er_loc[:, blk:blk + 1],
                                       in_=e_ps[:])
                r_loc = rpool2.tile([BLK, nb], BF16)
                nc.scalar.activation(out=r_loc[:], in_=er_loc[:],
                                     func=mybir.ActivationFunctionType.Exp,
                                     scale=-(1.0 - NEG_SLOPE))
                rt_ps = rps.tile([nb, BLK], BF16)
                nc.tensor.transpose(out=rt_ps[:], in_=r_loc[:],
                                    identity=identb[:])
                rt_sb = rpool2.tile([nb, BLK], BF16)
                nc.vector.tensor_copy(out=rt_sb[:], in_=rt_ps[:])
                nc.sync.dma_start(
                    out=rrow.ap().rearrange("o (b f) -> (o b) f", b=nb),
                    in_=rt_sb[:])
                r_flat = rpool2.tile([1, core_nodes], BF16)
                nc.sync.dma_start(out=r_flat[:], in_=rrow[:])
                for j0 in range(0, core_nodes, 512):
                    j1 = min(j0 + 512, core_nodes)
                    rb_ps = rps.tile([128, j1 - j0], F32, tag="rb",
                                     name="rb_ps")
                    nc.tensor.matmul(out=rb_ps[:], lhsT=ones1[:],
                                     rhs=r_flat[:, j0:j1],
                                     start=True, stop=True)
                    nc.vector.tensor_copy(out=r_rep[:, j0:j1], in_=rb_ps[:])

            # ---- interleaved phase 1 (table build) + phase 2 (edges) ----
            with tc.tile_pool(name="ix", bufs=1) as ixpool:
                srcw_sb = ixpool.tile([BLK, NW], I16)
                nc.sync.dma_start(out=srcw_sb[:], in_=srcw[:])
                dl_sb = ixpool.tile([BLK, T], F32)
                nc.sync.dma_start(out=dl_sb[:], in_=dstloc[:])

                with tc.tile_pool(name="zh", bufs=3) as hpool, \
                     tc.tile_pool(name="zrow", bufs=1) as zrpool, \
                     tc.tile_pool(name="zps", bufs=2, space="PSUM") as zps, \
                     tc.tile_pool(name="zg", bufs=4) as zgpool, \
                     tc.tile_pool(name="ar", bufs=4) as arpool, \
                     tc.tile_pool(name="m", bufs=4) as mpool, \
                     tc.tile_pool(name="oh", bufs=4) as ohpool, \
                     tc.tile_pool(name="acc", bufs=1) as apool, \
                     tc.tile_pool(name="yps", bufs=4, space="PSUM") as ypool, \
                     tc.tile_pool(name="ytp", bufs=2, space="PSUM") as ytpool, \
                     tc.tile_pool(name="fin", bufs=6) as fpool, \
                     tc.tile_pool(name="ost", bufs=2) as opool:
                    NY = C_ONE + 1
                    # pinned zrows buffers: ones column set once
                    ZRB = 3
                    zrows_bufs = []
                    for i in range(ZRB):
                        zr = zrpool.tile([128, sub, EL], BF16, tag=f"zr{i}",
                                         name=f"zrows{i}")
                        if SIM_INIT:
                            nc.vector.memset(zr[:, :, C_R + 1:], 0.0)
                        nc.vector.memset(zr[:, :, C_ONE:C_ONE + 1], 1.0)
                        zrows_bufs.append(zr)
                    acc = apool.tile([NY, nb, BLK], F32)
                    nc.vector.memset(acc[:], 0.0)

                    def phase1_chunk(q):
                        zaug_t = zaug[q].ap().rearrange(
                            "(p c) z -> p c z", p=BLK)
                        for mm in range(gpc):
                            g = q * gpc + mm
                            n0 = g * ZG
                            htile = hpool.tile([IN_FEATS, ZG], BF16,
                                               tag="ht", name="htile")
                            nc.sync.dma_start(out=htile[:],
                                              in_=hT[:, n0:n0 + ZG])
                            zrows = zrows_bufs[g % ZRB]
                            for s in range(sub):
                                z_ps = zps.tile([128, C_ONE], F32, tag="zp",
                                                name="z_ps")
                                nc.tensor.matmul(
                                    out=z_ps[:],
                                    lhsT=htile[:, s * BLK:(s + 1) * BLK],
                                    rhs=waug[:], start=True, stop=True)
                                nc.scalar.copy(out=zrows[:, s, 0:C_ONE],
                                               in_=z_ps[:])
                            nc.scalar.activation(
                                out=zrows[:, :, C_A],
                                in_=zrows[:, :, C_EL],
                                func=mybir.ActivationFunctionType.Exp)
                            nc.scalar.activation(
                                out=zrows[:, :, C_R],
                                in_=zrows[:, :, C_EL],
                                func=mybir.ActivationFunctionType.Exp,
                                scale=-(1.0 - NEG_SLOPE))
                            if SIM_INIT:
                                nc.sync.dma_start(
                                    out=zaug_t[:, sub * mm:sub * (mm + 1), :],
                                    in_=zrows[:])
                            else:
                                # write only the 37 used columns of each
                                # 256B row (sliced rows cut HBM traffic 3.5x)
                                nc.sync.dma_start(
                                    out=zaug_t[:, sub * mm:sub * (mm + 1),
                                               0:C_R + 1],
                                    in_=zrows[:, :, 0:C_R + 1])

                    def phase2_chunk(q):
                        for bgi in range(NGB):
                            colbase = q * nb * C + bgi * NCOL
                            w0 = colbase * BLK // 16
                            zg = zgpool.tile([BLK, NCOL, EL], BF16,
                                             tag="zg", name="zg")
                            GCH = 8  # tile-columns per call (1024 idxs)
                            gr = range(0) if SKIP_GATHER else range(0, NCOL, GCH)
                            for j0 in gr:
                                j1 = min(j0 + GCH, NCOL)
                                ni = (j1 - j0) * BLK
                                wj = w0 + j0 * BLK // 16
                                nc.gpsimd.dma_gather(
                                    out_ap=zg[:, j0:j1, :],
                                    in_ap=zaug[q][:],
                                    idxs_ap=srcw_sb[:, wj:wj + ni // 16],
                                    num_idxs=ni, num_idxs_reg=ni,
                                    elem_size=EL,
                                    queue_num=_qrr())
                            if SKIP_GATHER:
                                nc.vector.memset(zg[:, 0:1, :], 0.0)
                            a_sb = arpool.tile([BLK, NCOL], F32, tag="a",
                                               name="a_sb")
                            nc.vector.tensor_copy(out=a_sb[:],
                                                  in_=zg[:, :, C_A])
                            rr_sb = arpool.tile([BLK, NCOL], F32, tag="rr",
                                                name="rr_sb")
                            nc.vector.tensor_copy(out=rr_sb[:],
                                                  in_=zg[:, :, C_R])
                            for p0 in range(0) if SKIP_P2C else range(0, bgs, 4):
                                pw = min(4, bgs - p0)
                                y_ps = ypool.tile([NY, 4, BLK], F32, tag="y",
                                                  name="y_ps")
                                for bi in range(pw):
                                    b = p0 + bi
                                    bb = bgi * bgs + b
                                    m_all = mpool.tile([BLK, C, BLK], BF16,
                                                       tag="m", name="m_t")
                                    oh_all = ohpool.tile([BLK, C, BLK], BF16,
                                                         tag="oh", name="oh")
                                    for t in range(C):
                                        lcol = b * C + t
                                        col = colbase + lcol
                                        nc.vector.tensor_scalar(
                                            oh_all[:, t, :], iota[:],
                                            dl_sb[:, col:col + 1],
                                            a_sb[:, lcol:lcol + 1],
                                            mybir.AluOpType.is_equal,
                                            mybir.AluOpType.mult)
                                        # m = oh * r_e * R_n (fused)
                                        nc.vector.scalar_tensor_tensor(
                                            out=m_all[:, t, :],
                                            in0=r_rep[:, bb * BLK:
                                                      (bb + 1) * BLK],
                                            scalar=rr_sb[:, lcol:lcol + 1],
                                            in1=oh_all[:, t, :],
                                            op0=mybir.AluOpType.mult,
                                            op1=mybir.AluOpType.mult)
                                    # oh = max(oh, oh*r*R) = oh*max(1, r*R)
                                    nc.vector.tensor_tensor(
                                        out=oh_all[:], in0=oh_all[:],
                                        in1=m_all[:],
                                        op=mybir.AluOpType.max)
                                    for t in range(C):
                                        lcol = b * C + t
                                        nc.tensor.matmul(
                                            out=y_ps[:, bi, :],
                                            lhsT=zg[:, lcol, 0:NY],
                                            rhs=oh_all[:, t, :],
                                            start=(t == 0),
                                            stop=(t == C - 1))
                                bb0 = bgi * bgs + p0
                                nc.vector.tensor_add(
                                    out=acc[:, bb0:bb0 + pw, :],
                                    in0=acc[:, bb0:bb0 + pw, :],
                                    in1=y_ps[:, 0:pw, :])

                    phase1_chunk(0)
                    for q in range(NQ):
                        if q + 1 < NQ:
                            phase1_chunk(q + 1)
                        phase2_chunk(q)

                    # ---- finalize: normalize and write out ----
                    OB = 14  # blocks per output group
                    for og in range(nb // OB):
                        ost = opool.tile([BLK, OB, OUT_FEATS], F32)
                        for b in range(OB):
                            bb = og * OB + b
                            yt = ytpool.tile([BLK, NY], F32)
                            nc.tensor.transpose(out=yt[:], in_=acc[:, bb, :],
                                                identity=ident[0:NY, 0:NY])
                            den = fpool.tile([BLK, 1], F32)
                            nc.vector.tensor_scalar(
                                den[:], yt[:, C_ONE:C_ONE + 1], 1e-16, None,
                                mybir.AluOpType.max)
                            rden = fpool.tile([BLK, 1], F32)
                            nc.vector.reciprocal(out=rden[:], in_=den[:])
                            nc.vector.tensor_scalar(
                                ost[:, b, :], yt[:, 0:OUT_FEATS], rden[:],
                                None, mybir.AluOpType.mult)
                        n0 = og * OB * BLK
                        nc.sync.dma_start(
                            out=out[n0:n0 + OB * BLK, :].rearrange(
                                "(s p) c -> p s c", p=BLK),
                            in_=ost[:])

    nc.compile()
    return nc


def _prep(h, W, a, src, dst, nb=NB, n_nodes=N_NODES):
    """Host-side sharding / index layout (integer index manipulation,
    zero-padding and dtype casts only - all floating-point math runs on
    device)."""
    core_nodes = nb * BLK
    npad = N_CORES * core_nodes
    chunk_nodes = npad // NQ
    chunk_cols = chunk_nodes // BLK

    h = np.asarray(h, dtype=np.float32)
    W = np.ascontiguousarray(np.asarray(W, dtype=np.float32))
    a = np.asarray(a, dtype=np.float32).reshape(-1)
    src = np.asarray(src, dtype=np.int64)
    dst = np.asarray(dst, dtype=np.int64)

    hT = np.zeros((IN_FEATS, npad), dtype=ml_dtypes.bfloat16)
    hT[:, :n_nodes] = h.T.astype(ml_dtypes.bfloat16)
    av = np.ascontiguousarray(a.reshape(-1, 1), dtype=np.float32)

    core = dst // core_nodes
    b_of = (dst % core_nodes) // BLK
    q_of = src // chunk_nodes
    grp = (core * NQ + q_of) * nb + b_of
    # chunk-local tiled table row of src
    loc = src - q_of * chunk_nodes
    src_t = (loc % BLK) * chunk_cols + loc // BLK
    # sort by (group, src-table-row): src-sorted runs improve gather locality
    order = np.argsort(grp * (1 << 24) + src_t, kind="stable")
    gs = grp[order]
    ds = dst[order]

    counts = np.bincount(gs, minlength=N_CORES * NQ * nb)
    C = int(max(1, -(-counts.max() // BLK)))
    T = NQ * nb * C
    NW = T * BLK // 16

    # global slot of each sorted edge
    starts = np.zeros(N_CORES * NQ * nb + 1, dtype=np.int64)
    np.cumsum(counts, out=starts[1:])
    rank = np.arange(len(gs)) - starts[gs]
    # within-core group index: (q * nb + b) for that core
    gloc = gs % (NQ * nb)
    slot = gloc * (C * BLK) + rank  # slot within the core's edge buffer

    src_i16 = src_t[order].astype(np.int16)
    dl_all = (ds % core_nodes - b_of[order] * BLK).astype(np.float32)

    srcw = np.zeros((N_CORES, BLK, NW), dtype=np.int16)
    dstloc = np.full((N_CORES, BLK, T), -1.0, dtype=np.float32)
    for k in range(N_CORES):
        m = core[order] == k
        sl = slot[m]
        sflat = np.zeros(T * BLK, dtype=np.int16)
        dflat = np.full(T * BLK, -1.0, dtype=np.float32)
        sflat[sl] = src_i16[m]
        dflat[sl] = dl_all[m]
        # wrapped-16, replicated over the 8 gpsimd groups
        srcw[k] = np.tile(sflat.reshape(-1, 16).T, (8, 1))
        dstloc[k] = dflat.reshape(T, BLK).T
    return hT, W, av, srcw, dstloc, C


def kernel(h, W, a, src, dst):
    hT, Wm, av, srcw, dstloc, C = _prep(h, W, a, src, dst)
    if C not in _cache:
        _cache[C] = _build(C)
    nc = _cache[C]
    in_maps = []
    for k in range(N_CORES):
        in_maps.append({
            "hT": hT,
            "Wt": Wm,
            "av": av,
            "srcw": srcw[k],
            "dstloc": dstloc[k],
        })
    global _last
    _last = (nc, in_maps)
    res = run_bass_kernel_spmd(nc, in_maps, list(range(N_CORES)))
    outs = [res.results[k]["out"] for k in range(N_CORES)]
    full = np.concatenate(outs, axis=0)[:N_NODES]
    return np.ascontiguousarray(full, dtype=np.float32)


_last = None


# revision 36
# speedup vs baseline: 1.0630x; 1.0428x over previous
"""GAT layer (single head) on 8 Trainium2 NeuronCores.

Strategy: destination-sharded edge parallelism, phase-pipelined.
  - Nodes padded to NPAD = 8*NB*128; core k owns NB blocks of 128 nodes.
  - Host sorts edges by (dst-core, src-chunk, dst-block, src-row) and pads
    each (block, chunk) run to whole tiles of 128 edges (capacity C tiles,
    the max over all runs). src-chunk = src // (NPAD/4) (column range), so
    the node table for chunk q is complete after the q-th quarter of
    phase 1; sorting by src-row within a run improves gather locality.
  - Device, per core:
      prepass: WAUG = [W.T | wl | wr]; er for the core's OWN nodes only
               (h slice @ wr), then R = exp(-0.8*er) broadcast to all
               partitions as a replicated block-major table R_rep.
      phase 1 (chunk q): zaug[n] = [z | el | er | 1 | A | r] (bf16,
               A = exp(el), r = exp(-0.8*el)) via one fused matmul with
               WAUG; 256B rows to the chunk-q DRAM table.
      phase 2 (chunk q): per edge tile of 128: dma_gather zaug[src]
               (4 SWDGE queues, 2048-descriptor ring). The per-dst-node
               factor exp(er) cancels in the softmax ratio, so
                   ex_eff[e, n] = A_e * max(1, r_e * R_n)
               equals exp(leaky_relu(el+er, 0.2)) / exp(er) exactly.
               Tile work (bf16): M = max(r_e*R_rep[b], 1);
               oh = (iota==dl)*A_e*M; Y[b] += [z|..|1].T @ oh in PSUM per
               (block, chunk); acc[b] += Y (f32, numerator rows 0:32,
               denominator row 34 via the ones column).
      Phase 1 chunk q+1 is emitted interleaved with phase 2 chunk q, so
      gathers and edge compute overlap table construction. Finally
      out = num / max(denom, eps) in f32. Softmax max-subtraction is
      dropped: |e| stays small for this model, so exp() is
      well-conditioned and the softmax ratio is unchanged.

  DRAM chunk tables use a tiled layout: node n (chunk-local l) lives at
  row (l % 128) * (NPAD/512) + l // 128 of table (n // (NPAD/4)), so
  phase 1 writes them with contiguous per-partition DMA runs; the host
  bakes this mapping into the gather indices.
"""

import sys

sys.path.insert(0, "/opt/trn_rl_repo")

import numpy as np
import ml_dtypes

import concourse.bacc as bacc
import concourse.bass as bass
import concourse.tile as tile
from concourse import mybir
from concourse.bass_utils import run_bass_kernel_spmd
from concourse.masks import make_identity

F32 = mybir.dt.float32
BF16 = mybir.dt.bfloat16
I16 = mybir.dt.int16

N_NODES = 100000
IN_FEATS = 128
OUT_FEATS = 32
NEG_SLOPE = 0.2
N_CORES = 8
BLK = 128
NB = 98  # blocks per core (full problem)
EL = 128  # table row: 128 bf16 = 256B (dma_gather granularity)
NQ = 4  # chunks of the z table (by node column range)
ZG = 512  # nodes per z-phase group
BGS = 7  # blocks per phase-2 gather group

C_EL = 32  # el column in zaug row
C_ER = 33  # er column
C_ONE = 34  # constant-one column
C_A = 35  # A = exp(el)
C_R = 36  # r = exp(-0.8*el)

_cache = {}
SIM_INIT = False  # set True when running under CoreSim (full-init for checker)
N_QUEUES = 4  # SWDGE queues to round-robin gathers over (1..4)
SCRATCH = 32768  # dynamic_dma_scratch_size (ring bytes; 16B/descriptor)
SKIP_GATHER = False  # timing probe: skip dma_gather (breaks correctness)
SKIP_P2C = False  # timing probe: skip phase-2 DVE/PE compute

_qctr = [0]


def _qrr():
    """Round-robin SWDGE queue assignment."""
    q = _qctr[0] % N_QUEUES
    _qctr[0] += 1
    return q


def _build(C, nb=NB, bgs=BGS):
    """C = tiles of 128 edges per (block, chunk) run."""
    assert nb % bgs == 0, (nb, bgs)
    core_nodes = nb * BLK
    npad = N_CORES * core_nodes
    ncols = npad // BLK
    chunk_nodes = npad // NQ
    chunk_cols = ncols // NQ  # 196
    chunk_rows = chunk_nodes  # rows per chunk table
    gpc = chunk_nodes // ZG  # phase-1 groups per chunk (49)
    sub = ZG // BLK
    assert chunk_rows < 32768 and core_nodes < 32768
    T = NQ * nb * C  # tile columns per core
    NW = T * BLK // 16  # wrapped-index columns
    NGB = nb // bgs  # gather groups per chunk (14)
    NCOL = bgs * C  # tile columns per gather group

    _qctr[0] = 0
    nc = bacc.Bacc("TRN2", target_bir_lowering=False, debug=False,
                   num_devices=N_CORES, num_swdge_queues=N_QUEUES,
                   dynamic_dma_scratch_size=SCRATCH)

    hT = nc.dram_tensor("hT", [IN_FEATS, npad], BF16, kind="ExternalInput")
    Wt = nc.dram_tensor("Wt", [OUT_FEATS, IN_FEATS], F32, kind="ExternalInput")
    av = nc.dram_tensor("av", [2 * OUT_FEATS, 1], F32, kind="ExternalInput")
    srcw = nc.dram_tensor("srcw", [BLK, NW], I16, kind="ExternalInput")
    dstloc = nc.dram_tensor("dstloc", [BLK, T], F32, kind="ExternalInput")
    out = nc.dram_tensor("out", [core_nodes, OUT_FEATS], F32,
                         kind="ExternalOutput")

    zaug = [nc.dram_tensor(f"zaug{q}", [chunk_rows, EL], BF16)
            for q in range(NQ)]
    rrow = nc.dram_tensor("rrow", [1, core_nodes], BF16)  # R flatten bounce

    hTv = hT.ap().rearrange("f (k n) -> f k n", k=N_CORES)

    with tile.TileContext(nc) as tc:
        with tc.tile_pool(name="const", bufs=1) as cpool:
            ident = cpool.tile([128, 128], F32)
            make_identity(nc, ident[:])
            identb = cpool.tile([128, 128], BF16)
            nc.vector.tensor_copy(out=identb[:], in_=ident[:])
            iota = cpool.tile([128, BLK], BF16)
            nc.gpsimd.iota(iota[:], pattern=[[1, BLK]], base=0,
                           channel_multiplier=0,
                           allow_small_or_imprecise_dtypes=True)
            ones1 = cpool.tile([1, BLK], BF16)
            nc.vector.memset(ones1[:], 1.0)

            # WAUG = [W.T | wl | wr]  (wl = W.T a_l, wr = W.T a_r)
            waug = cpool.tile([IN_FEATS, C_ONE], BF16)
            nc.vector.memset(waug[:], 0.0)
            with tc.tile_pool(name="wprep", bufs=1) as wpool, \
                 tc.tile_pool(name="wpsum", bufs=2, space="PSUM") as wps:
                w_sb = wpool.tile([OUT_FEATS, IN_FEATS], F32)
                nc.sync.dma_start(out=w_sb[:], in_=Wt[:])
                al_sb = wpool.tile([OUT_FEATS, 1], F32)
                nc.sync.dma_start(out=al_sb[:], in_=av[0:OUT_FEATS, :])
                ar_sb = wpool.tile([OUT_FEATS, 1], F32)
                nc.sync.dma_start(out=ar_sb[:],
                                  in_=av[OUT_FEATS:2 * OUT_FEATS, :])
                wt_ps = wps.tile([IN_FEATS, OUT_FEATS], F32)
                nc.tensor.transpose(out=wt_ps[:], in_=w_sb[:],
                                    identity=ident[0:OUT_FEATS, 0:OUT_FEATS])
                nc.vector.tensor_copy(out=waug[:, 0:OUT_FEATS], in_=wt_ps[:])
                wl_ps = wps.tile([IN_FEATS, 1], F32)
                nc.tensor.matmul(out=wl_ps[:], lhsT=w_sb[:],
                                 rhs=al_sb[:], start=True, stop=True)
                nc.vector.tensor_copy(out=waug[:, C_EL:C_EL + 1],
                                      in_=wl_ps[:])
                wr_ps = wps.tile([IN_FEATS, 1], F32)
                nc.tensor.matmul(out=wr_ps[:], lhsT=w_sb[:],
                                 rhs=ar_sb[:], start=True, stop=True)
                nc.vector.tensor_copy(out=waug[:, C_ER:C_ER + 1],
                                      in_=wr_ps[:])

            # ---- prepass: er for OWN nodes only -> R_rep (replicated) ----
            pid = nc.gpsimd.partition_id()
            r_rep = cpool.tile([128, nb * BLK], BF16)
            with tc.tile_pool(name="rprep", bufs=1) as rpool2, \
                 tc.tile_pool(name="rh", bufs=3) as rhpool, \
                 tc.tile_pool(name="rps", bufs=2, space="PSUM") as rps:
                er_loc = rpool2.tile([BLK, nb], F32)
                RZG = 7 * BLK  # 896 columns per own-h group (nb % 7 == 0)
                for j0 in range(0, core_nodes, RZG):
                    h2 = rhpool.tile([IN_FEATS, 1, RZG], BF16, tag="h2",
                                     name="h2")
                    nc.gpsimd.dma_start(
                        out=h2[:],
                        in_=hTv[:, bass.ts(pid, 1), j0:j0 + RZG])
                    for s in range(RZG // BLK):
                        blk = j0 // BLK + s
                        e_ps = rps.tile([BLK, 1], F32, tag="ep", name="e_ps")
                        nc.tensor.matmul(
                            out=e_ps[:],
                            lhsT=h2[:, 0, s * BLK:(s + 1) * BLK],
                            rhs=waug[:, C_ER:C_ER + 1],
                            start=True, stop=True)
                        nc.scalar.copy(out=er_loc[:, blk:blk + 1],
                                       in_=e_ps[:])
                r_loc = rpool2.tile([BLK, nb], BF16)
                nc.scalar.activation(out=r_loc[:], in_=er_loc[:],
                                     func=mybir.ActivationFunctionType.Exp,
                                     scale=-(1.0 - NEG_SLOPE))
                rt_ps = rps.tile([nb, BLK], BF16)
                nc.tensor.transpose(out=rt_ps[:], in_=r_loc[:],
                                    identity=identb[:])
                rt_sb = rpool2.tile([nb, BLK], BF16)
                nc.vector.tensor_copy(out=rt_sb[:], in_=rt_ps[:])
                nc.sync.dma_start(
                    out=rrow.ap().rearrange("o (b f) -> (o b) f", b=nb),
                    in_=rt_sb[:])
                r_flat = rpool2.tile([1, core_nodes], BF16)
                nc.sync.dma_start(out=r_flat[:], in_=rrow[:])
                for j0 in range(0, core_nodes, 512):
                    j1 = min(j0 + 512, core_nodes)
                    rb_ps = rps.tile([128, j1 - j0], F32, tag="rb",
                                     name="rb_ps")
                    nc.tensor.matmul(out=rb_ps[:], lhsT=ones1[:],
                                     rhs=r_flat[:, j0:j1],
                                     start=True, stop=True)
                    nc.vector.tensor_copy(out=r_rep[:, j0:j1], in_=rb_ps[:])

            # ---- interleaved phase 1 (table build) + phase 2 (edges) ----
            with tc.tile_pool(name="ix", bufs=1) as ixpool:
                srcw_sb = ixpool.tile([BLK, NW], I16)
                nc.sync.dma_start(out=srcw_sb[:], in_=srcw[:])
                dl_sb = ixpool.tile([BLK, T], F32)
                nc.sync.dma_start(out=dl_sb[:], in_=dstloc[:])

                with tc.tile_pool(name="zh", bufs=3) as hpool, \
                     tc.tile_pool(name="zrow", bufs=1) as zrpool, \
                     tc.tile_pool(name="zps", bufs=2, space="PSUM") as zps, \
                     tc.tile_pool(name="zg", bufs=4) as zgpool, \
                     tc.tile_pool(name="ar", bufs=4) as arpool, \
                     tc.tile_pool(name="m", bufs=4) as mpool, \
                     tc.tile_pool(name="oh", bufs=4) as ohpool, \
                     tc.tile_pool(name="acc", bufs=1) as apool, \
                     tc.tile_pool(name="yps", bufs=4, space="PSUM") as ypool, \
                     tc.tile_pool(name="ytp", bufs=2, space="PSUM") as ytpool, \
                     tc.tile_pool(name="fin", bufs=6) as fpool, \
                     tc.tile_pool(name="ost", bufs=2) as opool:
                    NY = C_ONE + 1
                    # pinned zrows buffers: ones column set once
                    ZRB = 3
                    zrows_bufs = []
                    for i in range(ZRB):
                        zr = zrpool.tile([128, sub, EL], BF16, tag=f"zr{i}",
                                         name=f"zrows{i}")
                        if SIM_INIT:
                            nc.vector.memset(zr[:, :, C_R + 1:], 0.0)
                        nc.vector.memset(zr[:, :, C_ONE:C_ONE + 1], 1.0)
                        zrows_bufs.append(zr)
                    acc = apool.tile([NY, nb, BLK], F32)
                    nc.vector.memset(acc[:], 0.0)

                    def phase1_chunk(q):
                        zaug_t = zaug[q].ap().rearrange(
                            "(p c) z -> p c z", p=BLK)
                        for mm in range(gpc):
                            g = q * gpc + mm
                            n0 = g * ZG
                            htile = hpool.tile([IN_FEATS, ZG], BF16,
                                               tag="ht", name="htile")
                            nc.sync.dma_start(out=htile[:],
                                              in_=hT[:, n0:n0 + ZG])
                            zrows = zrows_bufs[g % ZRB]
                            for s in range(sub):
                                z_ps = zps.tile([128, C_ONE], F32, tag="zp",
                                                name="z_ps")
                                nc.tensor.matmul(
                                    out=z_ps[:],
                                    lhsT=htile[:, s * BLK:(s + 1) * BLK],
                                    rhs=waug[:], start=True, stop=True)
                                nc.scalar.copy(out=zrows[:, s, 0:C_ONE],
                                               in_=z_ps[:])
                            nc.scalar.activation(
                                out=zrows[:, :, C_A],
                                in_=zrows[:, :, C_EL],
                                func=mybir.ActivationFunctionType.Exp)
                            nc.scalar.activation(
                                out=zrows[:, :, C_R],
                                in_=zrows[:, :, C_EL],
                                func=mybir.ActivationFunctionType.Exp,
                                scale=-(1.0 - NEG_SLOPE))
                            if SIM_INIT:
                                nc.sync.dma_start(
                                    out=zaug_t[:, sub * mm:sub * (mm + 1), :],
                                    in_=zrows[:])
                            else:
                                # write only the 37 used columns of each
                                # 256B row (sliced rows cut HBM traffic 3.5x)
                                nc.sync.dma_start(
                                    out=zaug_t[:, sub * mm:sub * (mm + 1),
                                               0:C_R + 1],
                                    in_=zrows[:, :, 0:C_R + 1])

                    def phase2_chunk(q):
                        for bgi in range(NGB):
                            colbase = q * nb * C + bgi * NCOL
                            w0 = colbase * BLK // 16
                            zg = zgpool.tile([BLK, NCOL, EL], BF16,
                                             tag="zg", name="zg")
                            GCH = 8  # tile-columns per call (1024 idxs)
                            gr = range(0) if SKIP_GATHER else range(0, NCOL, GCH)
                            for j0 in gr:
                                j1 = min(j0 + GCH, NCOL)
                                ni = (j1 - j0) * BLK
                                wj = w0 + j0 * BLK // 16
                                nc.gpsimd.dma_gather(
                                    out_ap=zg[:, j0:j1, :],
                                    in_ap=zaug[q][:],
                                    idxs_ap=srcw_sb[:, wj:wj + ni // 16],
                                    num_idxs=ni, num_idxs_reg=ni,
                                    elem_size=EL,
                                    queue_num=_qrr())
                            if SKIP_GATHER:
                                nc.vector.memset(zg[:, 0:1, :], 0.0)
                            a_sb = arpool.tile([BLK, NCOL], F32, tag="a",
                                               name="a_sb")
                            nc.vector.tensor_copy(out=a_sb[:],
                                                  in_=zg[:, :, C_A])
                            rr_sb = arpool.tile([BLK, NCOL], F32, tag="rr",
                                                name="rr_sb")
                            nc.vector.tensor_copy(out=rr_sb[:],
                                                  in_=zg[:, :, C_R])
                            for p0 in range(0) if SKIP_P2C else range(0, bgs, 4):
                                pw = min(4, bgs - p0)
                                y_ps = ypool.tile([NY, 4, BLK], F32, tag="y",
                                                  name="y_ps")
                                for bi in range(pw):
                                    b = p0 + bi
                                    bb = bgi * bgs + b
                                    m_all = mpool.tile([BLK, C, BLK], BF16,
                                                       tag="m", name="m_t")
                                    oh_all = ohpool.tile([BLK, C, BLK], BF16,
                                                         tag="oh", name="oh")
                                    for t in range(C):
                                        lcol = b * C + t
                                        col = colbase + lcol
                                        nc.vector.tensor_scalar(
                                            oh_all[:, t, :], iota[:],
                                            dl_sb[:, col:col + 1],
                                            a_sb[:, lcol:lcol + 1],
                                            mybir.AluOpType.is_equal,
                                            mybir.AluOpType.mult)
                                        # m = oh * r_e * R_n (fused)
                                        nc.vector.scalar_tensor_tensor(
                                            out=m_all[:, t, :],
                                            in0=r_rep[:, bb * BLK:
                                                      (bb + 1) * BLK],
                                            scalar=rr_sb[:, lcol:lcol + 1],
                                            in1=oh_all[:, t, :],
                                            op0=mybir.AluOpType.mult,
                                            op1=mybir.AluOpType.mult)
                                    # oh = max(oh, oh*r*R) = oh*max(1, r*R)
                                    nc.vector.tensor_tensor(
                                        out=oh_all[:], in0=oh_all[:],
                                        in1=m_all[:],
                                        op=mybir.AluOpType.max)
                                    for t in range(C):
                                        lcol = b * C + t
                                        nc.tensor.matmul(
                                            out=y_ps[:, bi, :],
                                            lhsT=zg[:, lcol, 0:NY],
                                            rhs=oh_all[:, t, :],
                                            start=(t == 0),
                                            stop=(t == C - 1))
                                bb0 = bgi * bgs + p0
                                nc.vector.tensor_add(
                                    out=acc[:, bb0:bb0 + pw, :],
                                    in0=acc[:, bb0:bb0 + pw, :],
                                    in1=y_ps[:, 0:pw, :])
                            if q == NQ - 1:
                                finalize_group(bgi)

                    def finalize_group(og):
                        # normalize + write out blocks [og*bgs, (og+1)*bgs);
                        # emitted per-group inside the last chunk so the
                        # output tail overlaps the remaining gathers
                        ost = opool.tile([BLK, bgs, OUT_FEATS], F32,
                                         tag="ost", name="ost")
                        for b in range(bgs):
                            bb = og * bgs + b
                            yt = ytpool.tile([BLK, NY], F32, tag="yt",
                                             name="yt")
                            nc.tensor.transpose(out=yt[:], in_=acc[:, bb, :],
                                                identity=ident[0:NY, 0:NY])
                            den = fpool.tile([BLK, 1], F32, tag="dn",
                                             name="den")
                            nc.vector.tensor_scalar(
                                den[:], yt[:, C_ONE:C_ONE + 1], 1e-16, None,
                                mybir.AluOpType.max)
                            rden = fpool.tile([BLK, 1], F32, tag="rd",
                                              name="rden")
                            nc.vector.reciprocal(out=rden[:], in_=den[:])
                            nc.vector.tensor_scalar(
                                ost[:, b, :], yt[:, 0:OUT_FEATS], rden[:],
                                None, mybir.AluOpType.mult)
                        n0 = og * bgs * BLK
                        nc.sync.dma_start(
                            out=out[n0:n0 + bgs * BLK, :].rearrange(
                                "(s p) c -> p s c", p=BLK),
                            in_=ost[:])

                    phase1_chunk(0)
                    for q in range(NQ):
                        if q + 1 < NQ:
                            phase1_chunk(q + 1)
                        phase2_chunk(q)

    nc.compile()
    return nc


def _prep(h, W, a, src, dst, nb=NB, n_nodes=N_NODES):
    """Host-side sharding / index layout (integer index manipulation,
    zero-padding and dtype casts only - all floating-point math runs on
    device)."""
    core_nodes = nb * BLK
    npad = N_CORES * core_nodes
    chunk_nodes = npad // NQ
    chunk_cols = chunk_nodes // BLK

    h = np.asarray(h, dtype=np.float32)
    W = np.ascontiguousarray(np.asarray(W, dtype=np.float32))
    a = np.asarray(a, dtype=np.float32).reshape(-1)
    src = np.asarray(src, dtype=np.int64)
    dst = np.asarray(dst, dtype=np.int64)

    hT = np.zeros((IN_FEATS, npad), dtype=ml_dtypes.bfloat16)
    hT[:, :n_nodes] = h.T.astype(ml_dtypes.bfloat16)
    av = np.ascontiguousarray(a.reshape(-1, 1), dtype=np.float32)

    core = dst // core_nodes
    b_of = (dst % core_nodes) // BLK
    q_of = src // chunk_nodes
    grp = (core * NQ + q_of) * nb + b_of
    # chunk-local tiled table row of src
    loc = src - q_of * chunk_nodes
    src_t = (loc % BLK) * chunk_cols + loc // BLK
    # sort by (group, src-table-row): src-sorted runs improve gather locality
    order = np.argsort(grp * (1 << 24) + src_t, kind="stable")
    gs = grp[order]
    ds = dst[order]

    counts = np.bincount(gs, minlength=N_CORES * NQ * nb)
    C = int(max(1, -(-counts.max() // BLK)))
    T = NQ * nb * C
    NW = T * BLK // 16

    # global slot of each sorted edge
    starts = np.zeros(N_CORES * NQ * nb + 1, dtype=np.int64)
    np.cumsum(counts, out=starts[1:])
    rank = np.arange(len(gs)) - starts[gs]
    # within-core group index: (q * nb + b) for that core
    gloc = gs % (NQ * nb)
    slot = gloc * (C * BLK) + rank  # slot within the core's edge buffer

    src_i16 = src_t[order].astype(np.int16)
    dl_all = (ds % core_nodes - b_of[order] * BLK).astype(np.float32)

    srcw = np.zeros((N_CORES, BLK, NW), dtype=np.int16)
    dstloc = np.full((N_CORES, BLK, T), -1.0, dtype=np.float32)
    for k in range(N_CORES):
        m = core[order] == k
        sl = slot[m]
        sflat = np.zeros(T * BLK, dtype=np.int16)
        dflat = np.full(T * BLK, -1.0, dtype=np.float32)
        sflat[sl] = src_i16[m]
        dflat[sl] = dl_all[m]
        # wrapped-16, replicated over the 8 gpsimd groups
        srcw[k] = np.tile(sflat.reshape(-1, 16).T, (8, 1))
        dstloc[k] = dflat.reshape(T, BLK).T
    return hT, W, av, srcw, dstloc, C


def kernel(h, W, a, src, dst):
    hT, Wm, av, srcw, dstloc, C = _prep(h, W, a, src, dst)
    if C not in _cache:
        _cache[C] = _build(C)
    nc = _cache[C]
    in_maps = []
    for k in range(N_CORES):
        in_maps.append({
            "hT": hT,
            "Wt": Wm,
            "av": av,
            "srcw": srcw[k],
            "dstloc": dstloc[k],
        })
    global _last
    _last = (nc, in_maps)
    res = run_bass_kernel_spmd(nc, in_maps, list(range(N_CORES)))
    outs = [res.results[k]["out"] for k in range(N_CORES)]
    full = np.concatenate(outs, axis=0)[:N_NODES]
    return np.ascontiguousarray(full, dtype=np.float32)


_last = None


# revision 38
# speedup vs baseline: 1.0900x; 1.0254x over previous
"""GAT layer (single head) on 8 Trainium2 NeuronCores.

Strategy: destination-sharded edge parallelism, phase-pipelined.
  - Nodes padded to NPAD = 8*NB*128; core k owns NB blocks of 128 nodes.
  - Host sorts edges by (dst-core, src-chunk, dst-block, src-row) and pads
    each (block, chunk) run to whole tiles of 128 edges (capacity C tiles,
    the max over all runs). src-chunk = src // (NPAD/4) (column range), so
    the node table for chunk q is complete after the q-th quarter of
    phase 1; sorting by src-row within a run improves gather locality.
  - Device, per core:
      prepass: WAUG = [W.T | wl | wr]; er for the core's OWN nodes only
               (h slice @ wr), then R = exp(-0.8*er) broadcast to all
               partitions as a replicated block-major table R_rep.
      phase 1 (chunk q): zaug[n] = [z | el | er | 1 | A | r] (bf16,
               A = exp(el), r = exp(-0.8*el)) via one fused matmul with
               WAUG; 256B rows to the chunk-q DRAM table.
      phase 2 (chunk q): per edge tile of 128: dma_gather zaug[src]
               (4 SWDGE queues, 2048-descriptor ring). The per-dst-node
               factor exp(er) cancels in the softmax ratio, so
                   ex_eff[e, n] = A_e * max(1, r_e * R_n)
               equals exp(leaky_relu(el+er, 0.2)) / exp(er) exactly.
               Tile work (bf16): M = max(r_e*R_rep[b], 1);
               oh = (iota==dl)*A_e*M; Y[b] += [z|..|1].T @ oh in PSUM per
               (block, chunk); acc[b] += Y (f32, numerator rows 0:32,
               denominator row 34 via the ones column).
      Phase 1 chunk q+1 is emitted interleaved with phase 2 chunk q, so
      gathers and edge compute overlap table construction. Finally
      out = num / max(denom, eps) in f32. Softmax max-subtraction is
      dropped: |e| stays small for this model, so exp() is
      well-conditioned and the softmax ratio is unchanged.

  DRAM chunk tables use a tiled layout: node n (chunk-local l) lives at
  row (l % 128) * (NPAD/512) + l // 128 of table (n // (NPAD/4)), so
  phase 1 writes them with contiguous per-partition DMA runs; the host
  bakes this mapping into the gather indices.
"""

import sys

sys.path.insert(0, "/opt/trn_rl_repo")

import numpy as np
import ml_dtypes

import concourse.bacc as bacc
import concourse.bass as bass
import concourse.tile as tile
from concourse import mybir
from concourse.bass_utils import run_bass_kernel_spmd
from concourse.masks import make_identity

F32 = mybir.dt.float32
BF16 = mybir.dt.bfloat16
I16 = mybir.dt.int16

N_NODES = 100000
IN_FEATS = 128
OUT_FEATS = 32
NEG_SLOPE = 0.2
N_CORES = 8
BLK = 128
NB = 98  # blocks per core (full problem)
EL = 128  # table row: 128 bf16 = 256B (dma_gather granularity)
NQ = 4  # chunks of the z table (by node column range)
ZG = 512  # nodes per z-phase group
BGS = 7  # blocks per phase-2 gather group

C_EL = 32  # el column in zaug row
C_ER = 33  # er column
C_ONE = 34  # constant-one column
C_A = 35  # A = exp(el)
C_R = 36  # r = exp(-0.8*el)

_cache = {}
SIM_INIT = False  # set True when running under CoreSim (full-init for checker)
N_QUEUES = 4  # SWDGE queues to round-robin gathers over (1..4)
SCRATCH = 32768  # dynamic_dma_scratch_size (ring bytes; 16B/descriptor)
SKIP_GATHER = False  # timing probe: skip dma_gather (breaks correctness)
SKIP_P2C = False  # timing probe: skip phase-2 DVE/PE compute

_qctr = [0]


def _qrr():
    """Round-robin SWDGE queue assignment."""
    q = _qctr[0] % N_QUEUES
    _qctr[0] += 1
    return q


def _build(C, nb=NB, bgs=BGS):
    """C = tiles of 128 edges per (block, chunk) run."""
    assert nb % bgs == 0, (nb, bgs)
    core_nodes = nb * BLK
    npad = N_CORES * core_nodes
    ncols = npad // BLK
    chunk_nodes = npad // NQ
    chunk_cols = ncols // NQ  # 196
    chunk_rows = chunk_nodes  # rows per chunk table
    gpc = chunk_nodes // ZG  # phase-1 groups per chunk (49)
    sub = ZG // BLK
    assert chunk_rows < 32768 and core_nodes < 32768
    T = NQ * nb * C  # tile columns per core
    NW = T * BLK // 16  # wrapped-index columns
    NGB = nb // bgs  # gather groups per chunk (14)
    NCOL = bgs * C  # tile columns per gather group

    _qctr[0] = 0
    nc = bacc.Bacc("TRN2", target_bir_lowering=False, debug=False,
                   num_devices=N_CORES, num_swdge_queues=N_QUEUES,
                   dynamic_dma_scratch_size=SCRATCH)

    hT = nc.dram_tensor("hT", [IN_FEATS, npad], BF16, kind="ExternalInput")
    Wt = nc.dram_tensor("Wt", [OUT_FEATS, IN_FEATS], F32, kind="ExternalInput")
    av = nc.dram_tensor("av", [2 * OUT_FEATS, 1], F32, kind="ExternalInput")
    srcw = nc.dram_tensor("srcw", [BLK, NW], I16, kind="ExternalInput")
    dstloc = nc.dram_tensor("dstloc", [BLK, T], F32, kind="ExternalInput")
    out = nc.dram_tensor("out", [core_nodes, OUT_FEATS], F32,
                         kind="ExternalOutput")

    zaug = [nc.dram_tensor(f"zaug{q}", [chunk_rows, EL], BF16)
            for q in range(NQ)]
    rrow = nc.dram_tensor("rrow", [1, core_nodes], BF16)  # R flatten bounce

    hTv = hT.ap().rearrange("f (k n) -> f k n", k=N_CORES)

    with tile.TileContext(nc) as tc:
        with tc.tile_pool(name="const", bufs=1) as cpool:
            ident = cpool.tile([128, 128], F32)
            make_identity(nc, ident[:])
            identb = cpool.tile([128, 128], BF16)
            nc.vector.tensor_copy(out=identb[:], in_=ident[:])
            iota = cpool.tile([128, BLK], BF16)
            nc.gpsimd.iota(iota[:], pattern=[[1, BLK]], base=0,
                           channel_multiplier=0,
                           allow_small_or_imprecise_dtypes=True)
            ones1 = cpool.tile([1, BLK], BF16)
            nc.vector.memset(ones1[:], 1.0)

            # WAUG = [W.T | wl | wr]  (wl = W.T a_l, wr = W.T a_r)
            waug = cpool.tile([IN_FEATS, C_ONE], BF16)
            nc.vector.memset(waug[:], 0.0)
            with tc.tile_pool(name="wprep", bufs=1) as wpool, \
                 tc.tile_pool(name="wpsum", bufs=2, space="PSUM") as wps:
                w_sb = wpool.tile([OUT_FEATS, IN_FEATS], F32)
                nc.sync.dma_start(out=w_sb[:], in_=Wt[:])
                al_sb = wpool.tile([OUT_FEATS, 1], F32)
                nc.sync.dma_start(out=al_sb[:], in_=av[0:OUT_FEATS, :])
                ar_sb = wpool.tile([OUT_FEATS, 1], F32)
                nc.sync.dma_start(out=ar_sb[:],
                                  in_=av[OUT_FEATS:2 * OUT_FEATS, :])
                wt_ps = wps.tile([IN_FEATS, OUT_FEATS], F32)
                nc.tensor.transpose(out=wt_ps[:], in_=w_sb[:],
                                    identity=ident[0:OUT_FEATS, 0:OUT_FEATS])
                nc.vector.tensor_copy(out=waug[:, 0:OUT_FEATS], in_=wt_ps[:])
                wl_ps = wps.tile([IN_FEATS, 1], F32)
                nc.tensor.matmul(out=wl_ps[:], lhsT=w_sb[:],
                                 rhs=al_sb[:], start=True, stop=True)
                nc.vector.tensor_copy(out=waug[:, C_EL:C_EL + 1],
                                      in_=wl_ps[:])
                wr_ps = wps.tile([IN_FEATS, 1], F32)
                nc.tensor.matmul(out=wr_ps[:], lhsT=w_sb[:],
                                 rhs=ar_sb[:], start=True, stop=True)
                nc.vector.tensor_copy(out=waug[:, C_ER:C_ER + 1],
                                      in_=wr_ps[:])

            # ---- prepass: er for OWN nodes only -> R_rep (replicated) ----
            pid = nc.gpsimd.partition_id()
            r_rep = cpool.tile([128, nb * BLK], BF16)
            with tc.tile_pool(name="rprep", bufs=1) as rpool2, \
                 tc.tile_pool(name="rh", bufs=3) as rhpool, \
                 tc.tile_pool(name="rps", bufs=2, space="PSUM") as rps:
                er_loc = rpool2.tile([BLK, nb], F32)
                RZG = 7 * BLK  # 896 columns per own-h group (nb % 7 == 0)
                for j0 in range(0, core_nodes, RZG):
                    h2 = rhpool.tile([IN_FEATS, 1, RZG], BF16, tag="h2",
                                     name="h2")
                    nc.gpsimd.dma_start(
                        out=h2[:],
                        in_=hTv[:, bass.ts(pid, 1), j0:j0 + RZG])
                    for s in range(RZG // BLK):
                        blk = j0 // BLK + s
                        e_ps = rps.tile([BLK, 1], F32, tag="ep", name="e_ps")
                        nc.tensor.matmul(
                            out=e_ps[:],
                            lhsT=h2[:, 0, s * BLK:(s + 1) * BLK],
                            rhs=waug[:, C_ER:C_ER + 1],
                            start=True, stop=True)
                        nc.scalar.copy(out=er_loc[:, blk:blk + 1],
                                       in_=e_ps[:])
                r_loc = rpool2.tile([BLK, nb], BF16)
                nc.scalar.activation(out=r_loc[:], in_=er_loc[:],
                                     func=mybir.ActivationFunctionType.Exp,
                                     scale=-(1.0 - NEG_SLOPE))
                rt_ps = rps.tile([nb, BLK], BF16)
                nc.tensor.transpose(out=rt_ps[:], in_=r_loc[:],
                                    identity=identb[:])
                rt_sb = rpool2.tile([nb, BLK], BF16)
                nc.vector.tensor_copy(out=rt_sb[:], in_=rt_ps[:])
                nc.sync.dma_start(
                    out=rrow.ap().rearrange("o (b f) -> (o b) f", b=nb),
                    in_=rt_sb[:])
                r_flat = rpool2.tile([1, core_nodes], BF16)
                nc.sync.dma_start(out=r_flat[:], in_=rrow[:])
                for j0 in range(0, core_nodes, 512):
                    j1 = min(j0 + 512, core_nodes)
                    rb_ps = rps.tile([128, j1 - j0], F32, tag="rb",
                                     name="rb_ps")
                    nc.tensor.matmul(out=rb_ps[:], lhsT=ones1[:],
                                     rhs=r_flat[:, j0:j1],
                                     start=True, stop=True)
                    nc.vector.tensor_copy(out=r_rep[:, j0:j1], in_=rb_ps[:])

            # ---- interleaved phase 1 (table build) + phase 2 (edges) ----
            with tc.tile_pool(name="ix", bufs=1) as ixpool:
                srcw_sb = ixpool.tile([BLK, NW], I16)
                nc.sync.dma_start(out=srcw_sb[:], in_=srcw[:])
                dl_sb = ixpool.tile([BLK, T], F32)
                nc.sync.dma_start(out=dl_sb[:], in_=dstloc[:])

                with tc.tile_pool(name="zh", bufs=3) as hpool, \
                     tc.tile_pool(name="zrow", bufs=1) as zrpool, \
                     tc.tile_pool(name="zps", bufs=2, space="PSUM") as zps, \
                     tc.tile_pool(name="zg", bufs=4) as zgpool, \
                     tc.tile_pool(name="ar", bufs=4) as arpool, \
                     tc.tile_pool(name="m", bufs=4) as mpool, \
                     tc.tile_pool(name="oh", bufs=4) as ohpool, \
                     tc.tile_pool(name="acc", bufs=1) as apool, \
                     tc.tile_pool(name="yps", bufs=4, space="PSUM") as ypool, \
                     tc.tile_pool(name="ytp", bufs=2, space="PSUM") as ytpool, \
                     tc.tile_pool(name="fin", bufs=6) as fpool, \
                     tc.tile_pool(name="ost", bufs=2) as opool:
                    NY = C_ONE + 1
                    # pinned zrows buffers: ones column set once
                    ZRB = 3
                    zrows_bufs = []
                    for i in range(ZRB):
                        zr = zrpool.tile([128, sub, EL], BF16, tag=f"zr{i}",
                                         name=f"zrows{i}")
                        if SIM_INIT:
                            nc.vector.memset(zr[:, :, C_R + 1:], 0.0)
                        nc.vector.memset(zr[:, :, C_ONE:C_ONE + 1], 1.0)
                        zrows_bufs.append(zr)
                    acc = apool.tile([NY, nb, BLK], F32)
                    nc.vector.memset(acc[:], 0.0)

                    def phase1_chunk(q):
                        zaug_t = zaug[q].ap().rearrange(
                            "(p c) z -> p c z", p=BLK)
                        for mm in range(gpc):
                            g = q * gpc + mm
                            n0 = g * ZG
                            htile = hpool.tile([IN_FEATS, ZG], BF16,
                                               tag="ht", name="htile")
                            nc.sync.dma_start(out=htile[:],
                                              in_=hT[:, n0:n0 + ZG])
                            zrows = zrows_bufs[g % ZRB]
                            for s in range(sub):
                                z_ps = zps.tile([128, C_ONE], F32, tag="zp",
                                                name="z_ps")
                                nc.tensor.matmul(
                                    out=z_ps[:],
                                    lhsT=htile[:, s * BLK:(s + 1) * BLK],
                                    rhs=waug[:], start=True, stop=True)
                                nc.scalar.copy(out=zrows[:, s, 0:C_ONE],
                                               in_=z_ps[:])
                            nc.scalar.activation(
                                out=zrows[:, :, C_A],
                                in_=zrows[:, :, C_EL],
                                func=mybir.ActivationFunctionType.Exp)
                            nc.scalar.activation(
                                out=zrows[:, :, C_R],
                                in_=zrows[:, :, C_EL],
                                func=mybir.ActivationFunctionType.Exp,
                                scale=-(1.0 - NEG_SLOPE))
                            if SIM_INIT:
                                nc.sync.dma_start(
                                    out=zaug_t[:, sub * mm:sub * (mm + 1), :],
                                    in_=zrows[:])
                            else:
                                # write only the 37 used columns of each
                                # 256B row (sliced rows cut HBM traffic 3.5x)
                                nc.sync.dma_start(
                                    out=zaug_t[:, sub * mm:sub * (mm + 1),
                                               0:C_R + 1],
                                    in_=zrows[:, :, 0:C_R + 1])

                    def phase2_chunk(q):
                        for bgi in range(NGB):
                            colbase = q * nb * C + bgi * NCOL
                            w0 = colbase * BLK // 16
                            zg = zgpool.tile([BLK, NCOL, EL], BF16,
                                             tag="zg", name="zg")
                            GCH = 8  # tile-columns per call (1024 idxs)
                            gr = range(0) if SKIP_GATHER else range(0, NCOL, GCH)
                            for j0 in gr:
                                j1 = min(j0 + GCH, NCOL)
                                ni = (j1 - j0) * BLK
                                wj = w0 + j0 * BLK // 16
                                nc.gpsimd.dma_gather(
                                    out_ap=zg[:, j0:j1, :],
                                    in_ap=zaug[q][:],
                                    idxs_ap=srcw_sb[:, wj:wj + ni // 16],
                                    num_idxs=ni, num_idxs_reg=ni,
                                    elem_size=EL,
                                    queue_num=_qrr())
                            if SKIP_GATHER:
                                nc.vector.memset(zg[:, 0:1, :], 0.0)
                            a_sb = arpool.tile([BLK, NCOL], F32, tag="a",
                                               name="a_sb")
                            nc.vector.tensor_copy(out=a_sb[:],
                                                  in_=zg[:, :, C_A])
                            rr_sb = arpool.tile([BLK, NCOL], F32, tag="rr",
                                                name="rr_sb")
                            nc.vector.tensor_copy(out=rr_sb[:],
                                                  in_=zg[:, :, C_R])
                            for p0 in range(0) if SKIP_P2C else range(0, bgs, 4):
                                pw = min(4, bgs - p0)
                                y_ps = ypool.tile([NY, 4, BLK], F32, tag="y",
                                                  name="y_ps")
                                for bi in range(pw):
                                    b = p0 + bi
                                    bb = bgi * bgs + b
                                    m_all = mpool.tile([BLK, C, BLK], BF16,
                                                       tag="m", name="m_t")
                                    oh_all = ohpool.tile([BLK, C, BLK], BF16,
                                                         tag="oh", name="oh")
                                    for t in range(C):
                                        lcol = b * C + t
                                        col = colbase + lcol
                                        nc.vector.tensor_scalar(
                                            oh_all[:, t, :], iota[:],
                                            dl_sb[:, col:col + 1],
                                            a_sb[:, lcol:lcol + 1],
                                            mybir.AluOpType.is_equal,
                                            mybir.AluOpType.mult)
                                        # m = oh * r_e * R_n (fused)
                                        nc.vector.scalar_tensor_tensor(
                                            out=m_all[:, t, :],
                                            in0=r_rep[:, bb * BLK:
                                                      (bb + 1) * BLK],
                                            scalar=rr_sb[:, lcol:lcol + 1],
                                            in1=oh_all[:, t, :],
                                            op0=mybir.AluOpType.mult,
                                            op1=mybir.AluOpType.mult)
                                    # oh = max(oh, oh*r*R) = oh*max(1, r*R)
                                    nc.vector.tensor_tensor(
                                        out=oh_all[:], in0=oh_all[:],
                                        in1=m_all[:],
                                        op=mybir.AluOpType.max)
                                    for t in range(C):
                                        lcol = b * C + t
                                        nc.tensor.matmul(
                                            out=y_ps[:, bi, :],
                                            lhsT=zg[:, lcol, 0:NY],
                                            rhs=oh_all[:, t, :],
                                            start=(t == 0),
                                            stop=(t == C - 1))
                                bb0 = bgi * bgs + p0
                                nc.vector.tensor_add(
                                    out=acc[:, bb0:bb0 + pw, :],
                                    in0=acc[:, bb0:bb0 + pw, :],
                                    in1=y_ps[:, 0:pw, :])
                            if q == NQ - 1:
                                finalize_group(bgi)

                    def finalize_group(og):
                        # normalize + write out blocks [og*bgs, (og+1)*bgs);
                        # emitted per-group inside the last chunk so the
                        # output tail overlaps the remaining gathers
                        ost = opool.tile([BLK, bgs, OUT_FEATS], F32,
                                         tag="ost", name="ost")
                        for b in range(bgs):
                            bb = og * bgs + b
                            yt = ytpool.tile([BLK, NY], F32, tag="yt",
                                             name="yt")
                            nc.tensor.transpose(out=yt[:], in_=acc[:, bb, :],
                                                identity=ident[0:NY, 0:NY])
                            den = fpool.tile([BLK, 1], F32, tag="dn",
                                             name="den")
                            nc.vector.tensor_scalar(
                                den[:], yt[:, C_ONE:C_ONE + 1], 1e-16, None,
                                mybir.AluOpType.max)
                            rden = fpool.tile([BLK, 1], F32, tag="rd",
                                              name="rden")
                            nc.vector.reciprocal(out=rden[:], in_=den[:])
                            nc.vector.tensor_scalar(
                                ost[:, b, :], yt[:, 0:OUT_FEATS], rden[:],
                                None, mybir.AluOpType.mult)
                        n0 = og * bgs * BLK
                        nc.sync.dma_start(
                            out=out[n0:n0 + bgs * BLK, :].rearrange(
                                "(s p) c -> p s c", p=BLK),
                            in_=ost[:])

                    phase1_chunk(0)
                    for q in range(NQ):
                        if q + 1 < NQ:
                            phase1_chunk(q + 1)
                        phase2_chunk(q)

    nc.compile()
    return nc


def _prep(h, W, a, src, dst, nb=NB, n_nodes=N_NODES):
    """Host-side sharding / index layout (integer index manipulation,
    zero-padding and dtype casts only - all floating-point math runs on
    device)."""
    core_nodes = nb * BLK
    npad = N_CORES * core_nodes
    chunk_nodes = npad // NQ
    chunk_cols = chunk_nodes // BLK

    h = np.asarray(h, dtype=np.float32)
    W = np.ascontiguousarray(np.asarray(W, dtype=np.float32))
    a = np.asarray(a, dtype=np.float32).reshape(-1)
    src = np.asarray(src, dtype=np.int64)
    dst = np.asarray(dst, dtype=np.int64)

    hT = np.zeros((IN_FEATS, npad), dtype=ml_dtypes.bfloat16)
    hT[:, :n_nodes] = h.T.astype(ml_dtypes.bfloat16)
    av = np.ascontiguousarray(a.reshape(-1, 1), dtype=np.float32)

    core = dst // core_nodes
    b_of = (dst % core_nodes) // BLK
    q_of = src // chunk_nodes
    grp = (core * NQ + q_of) * nb + b_of
    # chunk-local tiled table row of src
    loc = src - q_of * chunk_nodes
    src_t = (loc % BLK) * chunk_cols + loc // BLK
    # sort by (group, src-table-row): src-sorted runs improve gather locality
    order = np.argsort(grp * (1 << 24) + src_t, kind="stable")
    gs = grp[order]
    ds = dst[order]

    counts = np.bincount(gs, minlength=N_CORES * NQ * nb)
    C = int(max(1, -(-counts.max() // BLK)))
    T = NQ * nb * C
    NW = T * BLK // 16

    # global slot of each sorted edge
    starts = np.zeros(N_CORES * NQ * nb + 1, dtype=np.int64)
    np.cumsum(counts, out=starts[1:])
    rank = np.arange(len(gs)) - starts[gs]
    # within-core group index: (q * nb + b) for that core
    gloc = gs % (NQ * nb)
    slot = gloc * (C * BLK) + rank  # slot within the core's edge buffer

    src_i16 = src_t[order].astype(np.int16)
    dl_all = (ds % core_nodes - b_of[order] * BLK).astype(np.float32)

    srcw = np.zeros((N_CORES, BLK, NW), dtype=np.int16)
    dstloc = np.full((N_CORES, BLK, T), -1.0, dtype=np.float32)
    for k in range(N_CORES):
        m = core[order] == k
        sl = slot[m]
        sflat = np.zeros(T * BLK, dtype=np.int16)
        dflat = np.full(T * BLK, -1.0, dtype=np.float32)
        sflat[sl] = src_i16[m]
        dflat[sl] = dl_all[m]
        # wrapped-16, replicated over the 8 gpsimd groups
        srcw[k] = np.tile(sflat.reshape(-1, 16).T, (8, 1))
        dstloc[k] = dflat.reshape(T, BLK).T
    return hT, W, av, srcw, dstloc, C


def kernel(h, W, a, src, dst):
    hT, Wm, av, srcw, dstloc, C = _prep(h, W, a, src, dst)
    if C not in _cache:
        _cache[C] = _build(C)
    nc = _cache[C]
    in_maps = []
    for k in range(N_CORES):
        in_maps.append({
            "hT": hT,
            "Wt": Wm,
            "av": av,
            "srcw": srcw[k],
            "dstloc": dstloc[k],
        })
    global _last
    _last = (nc, in_maps)
    res = run_bass_kernel_spmd(nc, in_maps, list(range(N_CORES)))
    outs = [res.results[k]["out"] for k in range(N_CORES)]
    full = np.concatenate(outs, axis=0)[:N_NODES]
    return np.ascontiguousarray(full, dtype=np.float32)


_last = None


# revision 39
# speedup vs baseline: 1.1040x; 1.0129x over previous
"""GAT layer (single head) on 8 Trainium2 NeuronCores.

Strategy: destination-sharded edge parallelism, phase-pipelined.
  - Nodes padded to NPAD = 8*NB*128; core k owns NB blocks of 128 nodes.
  - Host sorts edges by (dst-core, src-chunk, dst-block, src-row) and pads
    each (block, chunk) run to whole tiles of 128 edges (capacity C tiles,
    the max over all runs). src-chunk = src // (NPAD/4) (column range), so
    the node table for chunk q is complete after the q-th quarter of
    phase 1; sorting by src-row within a run improves gather locality.
  - Device, per core:
      prepass: WAUG = [W.T | wl | wr]; er for the core's OWN nodes only
               (h slice @ wr), then R = exp(-0.8*er) broadcast to all
               partitions as a replicated block-major table R_rep.
      phase 1 (chunk q): zaug[n] = [z | el | er | 1 | A | r] (bf16,
               A = exp(el), r = exp(-0.8*el)) via one fused matmul with
               WAUG; 256B rows to the chunk-q DRAM table.
      phase 2 (chunk q): per edge tile of 128: dma_gather zaug[src]
               (4 SWDGE queues, 2048-descriptor ring). The per-dst-node
               factor exp(er) cancels in the softmax ratio, so
                   ex_eff[e, n] = A_e * max(1, r_e * R_n)
               equals exp(leaky_relu(el+er, 0.2)) / exp(er) exactly.
               Tile work (bf16): M = max(r_e*R_rep[b], 1);
               oh = (iota==dl)*A_e*M; Y[b] += [z|..|1].T @ oh in PSUM per
               (block, chunk); acc[b] += Y (f32, numerator rows 0:32,
               denominator row 34 via the ones column).
      Phase 1 chunk q+1 is emitted interleaved with phase 2 chunk q, so
      gathers and edge compute overlap table construction. Finally
      out = num / max(denom, eps) in f32. Softmax max-subtraction is
      dropped: |e| stays small for this model, so exp() is
      well-conditioned and the softmax ratio is unchanged.

  DRAM chunk tables use a tiled layout: node n (chunk-local l) lives at
  row (l % 128) * (NPAD/512) + l // 128 of table (n // (NPAD/4)), so
  phase 1 writes them with contiguous per-partition DMA runs; the host
  bakes this mapping into the gather indices.
"""

import sys

sys.path.insert(0, "/opt/trn_rl_repo")

import numpy as np
import ml_dtypes

import concourse.bacc as bacc
import concourse.bass as bass
import concourse.tile as tile
from concourse import mybir
from concourse.bass_utils import run_bass_kernel_spmd
from concourse.masks import make_identity

F32 = mybir.dt.float32
BF16 = mybir.dt.bfloat16
I16 = mybir.dt.int16

N_NODES = 100000
IN_FEATS = 128
OUT_FEATS = 32
NEG_SLOPE = 0.2
N_CORES = 8
BLK = 128
NB = 98  # blocks per core (full problem)
EL = 128  # table row: 128 bf16 = 256B (dma_gather granularity)
NQ = 4  # chunks of the z table (by node column range)
ZG = 512  # nodes per z-phase group
BGS = 14  # blocks per phase-2 gather group

C_EL = 32  # el column in zaug row
C_ER = 33  # er column
C_ONE = 34  # constant-one column
C_A = 35  # A = exp(el)
C_R = 36  # r = exp(-0.8*el)

_cache = {}
SIM_INIT = False  # set True when running under CoreSim (full-init for checker)
N_QUEUES = 4  # SWDGE queues to round-robin gathers over (1..4)
SCRATCH = 32768  # dynamic_dma_scratch_size (ring bytes; 16B/descriptor)
SKIP_GATHER = False  # timing probe: skip dma_gather (breaks correctness)
SKIP_P2C = False  # timing probe: skip phase-2 DVE/PE compute

_qctr = [0]


def _qrr():
    """Round-robin SWDGE queue assignment."""
    q = _qctr[0] % N_QUEUES
    _qctr[0] += 1
    return q


def _build(C, nb=NB, bgs=BGS):
    """C = tiles of 128 edges per (block, chunk) run."""
    assert nb % bgs == 0, (nb, bgs)
    core_nodes = nb * BLK
    npad = N_CORES * core_nodes
    ncols = npad // BLK
    chunk_nodes = npad // NQ
    chunk_cols = ncols // NQ  # 196
    chunk_rows = chunk_nodes  # rows per chunk table
    gpc = chunk_nodes // ZG  # phase-1 groups per chunk (49)
    sub = ZG // BLK
    assert chunk_rows < 32768 and core_nodes < 32768
    T = NQ * nb * C  # tile columns per core
    NW = T * BLK // 16  # wrapped-index columns
    NGB = nb // bgs  # gather groups per chunk (14)
    NCOL = bgs * C  # tile columns per gather group

    _qctr[0] = 0
    nc = bacc.Bacc("TRN2", target_bir_lowering=False, debug=False,
                   num_devices=N_CORES, num_swdge_queues=N_QUEUES,
                   dynamic_dma_scratch_size=SCRATCH)

    hT = nc.dram_tensor("hT", [IN_FEATS, npad], BF16, kind="ExternalInput")
    Wt = nc.dram_tensor("Wt", [OUT_FEATS, IN_FEATS], F32, kind="ExternalInput")
    av = nc.dram_tensor("av", [2 * OUT_FEATS, 1], F32, kind="ExternalInput")
    srcw = nc.dram_tensor("srcw", [BLK, NW], I16, kind="ExternalInput")
    dstloc = nc.dram_tensor("dstloc", [BLK, T], F32, kind="ExternalInput")
    out = nc.dram_tensor("out", [core_nodes, OUT_FEATS], F32,
                         kind="ExternalOutput")

    zaug = [nc.dram_tensor(f"zaug{q}", [chunk_rows, EL], BF16)
            for q in range(NQ)]
    rrow = nc.dram_tensor("rrow", [1, core_nodes], BF16)  # R flatten bounce

    hTv = hT.ap().rearrange("f (k n) -> f k n", k=N_CORES)

    with tile.TileContext(nc) as tc:
        with tc.tile_pool(name="const", bufs=1) as cpool:
            ident = cpool.tile([128, 128], F32)
            make_identity(nc, ident[:])
            identb = cpool.tile([128, 128], BF16)
            nc.vector.tensor_copy(out=identb[:], in_=ident[:])
            iota = cpool.tile([128, BLK], BF16)
            nc.gpsimd.iota(iota[:], pattern=[[1, BLK]], base=0,
                           channel_multiplier=0,
                           allow_small_or_imprecise_dtypes=True)
            ones1 = cpool.tile([1, BLK], BF16)
            nc.vector.memset(ones1[:], 1.0)

            # WAUG = [W.T | wl | wr]  (wl = W.T a_l, wr = W.T a_r)
            waug = cpool.tile([IN_FEATS, C_ONE], BF16)
            nc.vector.memset(waug[:], 0.0)
            with tc.tile_pool(name="wprep", bufs=1) as wpool, \
                 tc.tile_pool(name="wpsum", bufs=2, space="PSUM") as wps:
                w_sb = wpool.tile([OUT_FEATS, IN_FEATS], F32)
                nc.sync.dma_start(out=w_sb[:], in_=Wt[:])
                al_sb = wpool.tile([OUT_FEATS, 1], F32)
                nc.sync.dma_start(out=al_sb[:], in_=av[0:OUT_FEATS, :])
                ar_sb = wpool.tile([OUT_FEATS, 1], F32)
                nc.sync.dma_start(out=ar_sb[:],
                                  in_=av[OUT_FEATS:2 * OUT_FEATS, :])
                wt_ps = wps.tile([IN_FEATS, OUT_FEATS], F32)
                nc.tensor.transpose(out=wt_ps[:], in_=w_sb[:],
                                    identity=ident[0:OUT_FEATS, 0:OUT_FEATS])
                nc.vector.tensor_copy(out=waug[:, 0:OUT_FEATS], in_=wt_ps[:])
                wl_ps = wps.tile([IN_FEATS, 1], F32)
                nc.tensor.matmul(out=wl_ps[:], lhsT=w_sb[:],
                                 rhs=al_sb[:], start=True, stop=True)
                nc.vector.tensor_copy(out=waug[:, C_EL:C_EL + 1],
                                      in_=wl_ps[:])
                wr_ps = wps.tile([IN_FEATS, 1], F32)
                nc.tensor.matmul(out=wr_ps[:], lhsT=w_sb[:],
                                 rhs=ar_sb[:], start=True, stop=True)
                nc.vector.tensor_copy(out=waug[:, C_ER:C_ER + 1],
                                      in_=wr_ps[:])

            # ---- prepass: er for OWN nodes only -> R_rep (replicated) ----
            pid = nc.gpsimd.partition_id()
            r_rep = cpool.tile([128, nb * BLK], BF16)
            with tc.tile_pool(name="rprep", bufs=1) as rpool2, \
                 tc.tile_pool(name="rh", bufs=3) as rhpool, \
                 tc.tile_pool(name="rps", bufs=2, space="PSUM") as rps:
                er_loc = rpool2.tile([BLK, nb], F32)
                RZG = 7 * BLK  # 896 columns per own-h group (nb % 7 == 0)
                for j0 in range(0, core_nodes, RZG):
                    h2 = rhpool.tile([IN_FEATS, 1, RZG], BF16, tag="h2",
                                     name="h2")
                    nc.gpsimd.dma_start(
                        out=h2[:],
                        in_=hTv[:, bass.ts(pid, 1), j0:j0 + RZG])
                    for s in range(RZG // BLK):
                        blk = j0 // BLK + s
                        e_ps = rps.tile([BLK, 1], F32, tag="ep", name="e_ps")
                        nc.tensor.matmul(
                            out=e_ps[:],
                            lhsT=h2[:, 0, s * BLK:(s + 1) * BLK],
                            rhs=waug[:, C_ER:C_ER + 1],
                            start=True, stop=True)
                        nc.scalar.copy(out=er_loc[:, blk:blk + 1],
                                       in_=e_ps[:])
                r_loc = rpool2.tile([BLK, nb], BF16)
                nc.scalar.activation(out=r_loc[:], in_=er_loc[:],
                                     func=mybir.ActivationFunctionType.Exp,
                                     scale=-(1.0 - NEG_SLOPE))
                rt_ps = rps.tile([nb, BLK], BF16)
                nc.tensor.transpose(out=rt_ps[:], in_=r_loc[:],
                                    identity=identb[:])
                rt_sb = rpool2.tile([nb, BLK], BF16)
                nc.vector.tensor_copy(out=rt_sb[:], in_=rt_ps[:])
                nc.sync.dma_start(
                    out=rrow.ap().rearrange("o (b f) -> (o b) f", b=nb),
                    in_=rt_sb[:])
                r_flat = rpool2.tile([1, core_nodes], BF16)
                nc.sync.dma_start(out=r_flat[:], in_=rrow[:])
                for j0 in range(0, core_nodes, 512):
                    j1 = min(j0 + 512, core_nodes)
                    rb_ps = rps.tile([128, j1 - j0], F32, tag="rb",
                                     name="rb_ps")
                    nc.tensor.matmul(out=rb_ps[:], lhsT=ones1[:],
                                     rhs=r_flat[:, j0:j1],
                                     start=True, stop=True)
                    nc.vector.tensor_copy(out=r_rep[:, j0:j1], in_=rb_ps[:])

            # ---- interleaved phase 1 (table build) + phase 2 (edges) ----
            with tc.tile_pool(name="ix", bufs=1) as ixpool:
                srcw_sb = ixpool.tile([BLK, NW], I16)
                nc.sync.dma_start(out=srcw_sb[:], in_=srcw[:])
                dl_sb = ixpool.tile([BLK, T], F32)
                nc.sync.dma_start(out=dl_sb[:], in_=dstloc[:])

                with tc.tile_pool(name="zh", bufs=3) as hpool, \
                     tc.tile_pool(name="zrow", bufs=1) as zrpool, \
                     tc.tile_pool(name="zps", bufs=2, space="PSUM") as zps, \
                     tc.tile_pool(name="zg", bufs=3) as zgpool, \
                     tc.tile_pool(name="ar", bufs=4) as arpool, \
                     tc.tile_pool(name="m", bufs=4) as mpool, \
                     tc.tile_pool(name="oh", bufs=4) as ohpool, \
                     tc.tile_pool(name="acc", bufs=1) as apool, \
                     tc.tile_pool(name="yps", bufs=4, space="PSUM") as ypool, \
                     tc.tile_pool(name="ytp", bufs=2, space="PSUM") as ytpool, \
                     tc.tile_pool(name="fin", bufs=6) as fpool, \
                     tc.tile_pool(name="ost", bufs=2) as opool:
                    NY = C_ONE + 1
                    # pinned zrows buffers: ones column set once
                    ZRB = 3
                    zrows_bufs = []
                    for i in range(ZRB):
                        zr = zrpool.tile([128, sub, EL], BF16, tag=f"zr{i}",
                                         name=f"zrows{i}")
                        if SIM_INIT:
                            nc.vector.memset(zr[:, :, C_R + 1:], 0.0)
                        nc.vector.memset(zr[:, :, C_ONE:C_ONE + 1], 1.0)
                        zrows_bufs.append(zr)
                    acc = apool.tile([NY, nb, BLK], F32)
                    nc.vector.memset(acc[:], 0.0)

                    def phase1_chunk(q):
                        zaug_t = zaug[q].ap().rearrange(
                            "(p c) z -> p c z", p=BLK)
                        for mm in range(gpc):
                            g = q * gpc + mm
                            n0 = g * ZG
                            htile = hpool.tile([IN_FEATS, ZG], BF16,
                                               tag="ht", name="htile")
                            nc.sync.dma_start(out=htile[:],
                                              in_=hT[:, n0:n0 + ZG])
                            zrows = zrows_bufs[g % ZRB]
                            for s in range(sub):
                                z_ps = zps.tile([128, C_ONE], F32, tag="zp",
                                                name="z_ps")
                                nc.tensor.matmul(
                                    out=z_ps[:],
                                    lhsT=htile[:, s * BLK:(s + 1) * BLK],
                                    rhs=waug[:], start=True, stop=True)
                                nc.scalar.copy(out=zrows[:, s, 0:C_ONE],
                                               in_=z_ps[:])
                            nc.scalar.activation(
                                out=zrows[:, :, C_A],
                                in_=zrows[:, :, C_EL],
                                func=mybir.ActivationFunctionType.Exp)
                            nc.scalar.activation(
                                out=zrows[:, :, C_R],
                                in_=zrows[:, :, C_EL],
                                func=mybir.ActivationFunctionType.Exp,
                                scale=-(1.0 - NEG_SLOPE))
                            if SIM_INIT:
                                nc.sync.dma_start(
                                    out=zaug_t[:, sub * mm:sub * (mm + 1), :],
                                    in_=zrows[:])
                            else:
                                # write only the 37 used columns of each
                                # 256B row (sliced rows cut HBM traffic 3.5x)
                                nc.sync.dma_start(
                                    out=zaug_t[:, sub * mm:sub * (mm + 1),
                                               0:C_R + 1],
                                    in_=zrows[:, :, 0:C_R + 1])

                    def phase2_chunk(q):
                        for bgi in range(NGB):
                            colbase = q * nb * C + bgi * NCOL
                            w0 = colbase * BLK // 16
                            zg = zgpool.tile([BLK, NCOL, EL], BF16,
                                             tag="zg", name="zg")
                            GCH = 8  # tile-columns per call (1024 idxs)
                            gr = range(0) if SKIP_GATHER else range(0, NCOL, GCH)
                            for j0 in gr:
                                j1 = min(j0 + GCH, NCOL)
                                ni = (j1 - j0) * BLK
                                wj = w0 + j0 * BLK // 16
                                nc.gpsimd.dma_gather(
                                    out_ap=zg[:, j0:j1, :],
                                    in_ap=zaug[q][:],
                                    idxs_ap=srcw_sb[:, wj:wj + ni // 16],
                                    num_idxs=ni, num_idxs_reg=ni,
                                    elem_size=EL,
                                    queue_num=_qrr())
                            if SKIP_GATHER:
                                nc.vector.memset(zg[:, 0:1, :], 0.0)
                            a_sb = arpool.tile([BLK, NCOL], F32, tag="a",
                                               name="a_sb")
                            nc.vector.tensor_copy(out=a_sb[:],
                                                  in_=zg[:, :, C_A])
                            rr_sb = arpool.tile([BLK, NCOL], F32, tag="rr",
                                                name="rr_sb")
                            nc.vector.tensor_copy(out=rr_sb[:],
                                                  in_=zg[:, :, C_R])
                            for p0 in range(0) if SKIP_P2C else range(0, bgs, 4):
                                pw = min(4, bgs - p0)
                                y_ps = ypool.tile([NY, 4, BLK], F32, tag="y",
                                                  name="y_ps")
                                for bi in range(pw):
                                    b = p0 + bi
                                    bb = bgi * bgs + b
                                    m_all = mpool.tile([BLK, C, BLK], BF16,
                                                       tag="m", name="m_t")
                                    oh_all = ohpool.tile([BLK, C, BLK], BF16,
                                                         tag="oh", name="oh")
                                    for t in range(C):
                                        lcol = b * C + t
                                        col = colbase + lcol
                                        nc.vector.tensor_scalar(
                                            oh_all[:, t, :], iota[:],
                                            dl_sb[:, col:col + 1],
                                            a_sb[:, lcol:lcol + 1],
                                            mybir.AluOpType.is_equal,
                                            mybir.AluOpType.mult)
                                        # m = oh * r_e * R_n (fused)
                                        nc.vector.scalar_tensor_tensor(
                                            out=m_all[:, t, :],
                                            in0=r_rep[:, bb * BLK:
                                                      (bb + 1) * BLK],
                                            scalar=rr_sb[:, lcol:lcol + 1],
                                            in1=oh_all[:, t, :],
                                            op0=mybir.AluOpType.mult,
                                            op1=mybir.AluOpType.mult)
                                    # oh = max(oh, oh*r*R) = oh*max(1, r*R)
                                    nc.vector.tensor_tensor(
                                        out=oh_all[:], in0=oh_all[:],
                                        in1=m_all[:],
                                        op=mybir.AluOpType.max)
                                    for t in range(C):
                                        lcol = b * C + t
                                        nc.tensor.matmul(
                                            out=y_ps[:, bi, :],
                                            lhsT=zg[:, lcol, 0:NY],
                                            rhs=oh_all[:, t, :],
                                            start=(t == 0),
                                            stop=(t == C - 1))
                                bb0 = bgi * bgs + p0
                                nc.vector.tensor_add(
                                    out=acc[:, bb0:bb0 + pw, :],
                                    in0=acc[:, bb0:bb0 + pw, :],
                                    in1=y_ps[:, 0:pw, :])
                            if q == NQ - 1:
                                finalize_group(bgi)

                    def finalize_group(og):
                        # normalize + write out blocks [og*bgs, (og+1)*bgs);
                        # emitted per-group inside the last chunk so the
                        # output tail overlaps the remaining gathers
                        ost = opool.tile([BLK, bgs, OUT_FEATS], F32,
                                         tag="ost", name="ost")
                        for b in range(bgs):
                            bb = og * bgs + b
                            yt = ytpool.tile([BLK, NY], F32, tag="yt",
                                             name="yt")
                            nc.tensor.transpose(out=yt[:], in_=acc[:, bb, :],
                                                identity=ident[0:NY, 0:NY])
                            den = fpool.tile([BLK, 1], F32, tag="dn",
                                             name="den")
                            nc.vector.tensor_scalar(
                                den[:], yt[:, C_ONE:C_ONE + 1], 1e-16, None,
                                mybir.AluOpType.max)
                            rden = fpool.tile([BLK, 1], F32, tag="rd",
                                              name="rden")
                            nc.vector.reciprocal(out=rden[:], in_=den[:])
                            nc.vector.tensor_scalar(
                                ost[:, b, :], yt[:, 0:OUT_FEATS], rden[:],
                                None, mybir.AluOpType.mult)
                        n0 = og * bgs * BLK
                        nc.sync.dma_start(
                            out=out[n0:n0 + bgs * BLK, :].rearrange(
                                "(s p) c -> p s c", p=BLK),
                            in_=ost[:])

                    phase1_chunk(0)
                    for q in range(NQ):
                        if q + 1 < NQ:
                            phase1_chunk(q + 1)
                        phase2_chunk(q)

    nc.compile()
    return nc


def _prep(h, W, a, src, dst, nb=NB, n_nodes=N_NODES):
    """Host-side sharding / index layout (integer index manipulation,
    zero-padding and dtype casts only - all floating-point math runs on
    device)."""
    core_nodes = nb * BLK
    npad = N_CORES * core_nodes
    chunk_nodes = npad // NQ
    chunk_cols = chunk_nodes // BLK

    h = np.asarray(h, dtype=np.float32)
    W = np.ascontiguousarray(np.asarray(W, dtype=np.float32))
    a = np.asarray(a, dtype=np.float32).reshape(-1)
    src = np.asarray(src, dtype=np.int64)
    dst = np.asarray(dst, dtype=np.int64)

    hT = np.zeros((IN_FEATS, npad), dtype=ml_dtypes.bfloat16)
    hT[:, :n_nodes] = h.T.astype(ml_dtypes.bfloat16)
    av = np.ascontiguousarray(a.reshape(-1, 1), dtype=np.float32)

    core = dst // core_nodes
    b_of = (dst % core_nodes) // BLK
    q_of = src // chunk_nodes
    grp = (core * NQ + q_of) * nb + b_of
    # chunk-local tiled table row of src
    loc = src - q_of * chunk_nodes
    src_t = (loc % BLK) * chunk_cols + loc // BLK
    # sort by (group, src-table-row): src-sorted runs improve gather locality
    order = np.argsort(grp * (1 << 24) + src_t, kind="stable")
    gs = grp[order]
    ds = dst[order]

    counts = np.bincount(gs, minlength=N_CORES * NQ * nb)
    C = int(max(1, -(-counts.max() // BLK)))
    T = NQ * nb * C
    NW = T * BLK // 16

    # global slot of each sorted edge
    starts = np.zeros(N_CORES * NQ * nb + 1, dtype=np.int64)
    np.cumsum(counts, out=starts[1:])
    rank = np.arange(len(gs)) - starts[gs]
    # within-core group index: (q * nb + b) for that core
    gloc = gs % (NQ * nb)
    slot = gloc * (C * BLK) + rank  # slot within the core's edge buffer

    src_i16 = src_t[order].astype(np.int16)
    dl_all = (ds % core_nodes - b_of[order] * BLK).astype(np.float32)

    srcw = np.zeros((N_CORES, BLK, NW), dtype=np.int16)
    dstloc = np.full((N_CORES, BLK, T), -1.0, dtype=np.float32)
    for k in range(N_CORES):
        m = core[order] == k
        sl = slot[m]
        sflat = np.zeros(T * BLK, dtype=np.int16)
        dflat = np.full(T * BLK, -1.0, dtype=np.float32)
        sflat[sl] = src_i16[m]
        dflat[sl] = dl_all[m]
        # wrapped-16, replicated over the 8 gpsimd groups
        srcw[k] = np.tile(sflat.reshape(-1, 16).T, (8, 1))
        dstloc[k] = dflat.reshape(T, BLK).T
    return hT, W, av, srcw, dstloc, C


def kernel(h, W, a, src, dst):
    hT, Wm, av, srcw, dstloc, C = _prep(h, W, a, src, dst)
    if C not in _cache:
        _cache[C] = _build(C)
    nc = _cache[C]
    in_maps = []
    for k in range(N_CORES):
        in_maps.append({
            "hT": hT,
            "Wt": Wm,
            "av": av,
            "srcw": srcw[k],
            "dstloc": dstloc[k],
        })
    global _last
    _last = (nc, in_maps)
    res = run_bass_kernel_spmd(nc, in_maps, list(range(N_CORES)))
    outs = [res.results[k]["out"] for k in range(N_CORES)]
    full = np.concatenate(outs, axis=0)[:N_NODES]
    return np.ascontiguousarray(full, dtype=np.float32)


_last = None
